# revision 1
# baseline (speedup 1.0000x reference)
"""GAT 2-layer kernel for 8 trn2 NeuronCores (self-contained).

Sharding: destination-node blocks across 8 cores. Per core: 49 blocks x 128
nodes. Layer GEMMs computed on owned nodes; per-node attention factors
(u=exp(a_src), v=exp(0.2 a_src), and dst-side) folded into gatherable row
tables (Hcat/Gcat) that are AllGathered; edge phases gather rows by src via
dma_gather and do segment-softmax-aggregation with per-chunk selection-matrix
matmuls (exp(leaky(x)) == max(exp(x), exp(0.2 x)) makes the logits separable).
"""
import sys, types
sys.path.insert(0, '/opt/trn_rl_repo')

import numpy as np
import ml_dtypes

# ---------------- problem constants (hardcoded) ----------------
N = 50000
F_IN = 512
HID = 64
HEADS = 8
CLS = 64
NEG_SLOPE = 0.2
NCORES = 8
NPC = 6250
NB = 49
BLK = 128
NLOC = NB * BLK           # 6272
NGLOB = NCORES * NLOC     # 50176
CPS = 6                   # L1 chunks per side per block
SIDE = CPS * 128          # 768 edge slots per side
L1SEG = [(0, 3), (3, 6)]  # gather segments (chunk ranges) per side
CH2 = 12                  # L2 chunks per block
SIDE2 = CH2 * 128         # 1536 L2 edge slots per block
L2SEG = [(0, 4), (4, 8), (8, 12)]
HROW = 640                # Hcat row (bf16)
GROW = 128                # Gcat row (bf16)
NHALF = NGLOB // 2        # 25088
L2Q = 4
EPS = 1e-16


def _install_ntff_hook():
    if 'antenv.axon_hooks' in sys.modules:
        return
    try:
        sys.path.insert(0, '/root/.axon_site')
        from trn_agent_boot.trn_boot import _ntff_profile_via_ctypes
        hook = _ntff_profile_via_ctypes('/opt/axon/libaxon_pjrt.so')
    except Exception:
        return
    mod = types.ModuleType('antenv.axon_hooks')
    mod._hook = hook
    mod.get_axon_ntff_profile_hook = lambda: mod._hook
    mod.set_axon_ntff_profile_hook = lambda h: setattr(mod, '_hook', h)
    sys.modules['antenv.axon_hooks'] = mod


_install_ntff_hook()

import concourse.bass as bass
import concourse.mybir as mybir
import concourse.tile as tile
from concourse import library_config
from concourse.bass_utils import run_bass_kernel_spmd
from concourse.vector_clock import VectorClock, ScopedClock

bf16 = mybir.dt.bfloat16
f32 = mybir.dt.float32

# ------------- tile framework patches (walrus: 1 sync wait / inst) ---------


def _drain_and_barrier_split(self, tick_clock, wait_clock):
    nc = self.nc
    full = tick_clock.global_clock
    procs = [p for p in range(27) if full[p] > 0]
    for p in procs:
        sub = VectorClock([full[q] if q == p else 0 for q in range(27)])
        drain_inst = nc.sync.drain(fusable=False)
        wait_clock.add_sem_waits(drain_inst.ins, ScopedClock({None: sub}))
    if not procs:
        nc.sync.drain(fusable=False)
    nc.all_engine_barrier()
    assert self.sems is not None
    popped = nc._tile_sem_poison_stack.pop()
    assert popped is self._sem_poison
    nc.clear_and_free_semaphores(list(self.sems.allocated().values()))
    nc.all_engine_barrier()


def _split_excess_waits(nc):
    for bb in nc.main_func.blocks:
        insts = bb.instructions
        i = 0
        while i < len(insts):
            ins = insts[i]
            si = ins.sync_info
            if si is None:
                i += 1
                continue
            waits = list(si.on_wait)
            if len(waits) <= 1:
                i += 1
                continue
            keep, surplus = waits[:1], waits[1:]
            ins.sync_info = mybir.SyncInfo(on_wait=keep, on_update=list(si.on_update))
            nops = []
            for w in surplus:
                nop = mybir.InstNoOp(name=nc.get_next_instruction_name())
                nop.engine = ins.engine
                nop.sync_info = mybir.SyncInfo(on_wait=[w], on_update=[])
                nc.register_instruction(nop)
                nops.append(nop)
            for k, nop in enumerate(nops):
                insts.insert(i + k, nop)
            i += 1 + len(nops)


_PATCHED = False


def _install_patches():
    global _PATCHED
    if _PATCHED:
        return
    _orig_exit = tile.TileContext.__exit__

    def _exit_with_split(self, *a):
        r = _orig_exit(self, *a)
        _split_excess_waits(self.nc)
        return r

    tile.TileContext._drain_and_barrier = _drain_and_barrier_split
    tile.TileContext.__exit__ = _exit_with_split
    _PATCHED = True


# ---------------- host-side graph preprocessing ----------------


def _wrap16(flat_idx):
    W = len(flat_idx) // 16
    t = np.asarray(flat_idx, np.int16).reshape(W, 16).T
    return np.tile(t, (8, 1))


def _host_prep(x, edge_index, W1, att_src1, att_dst1, b1, W2, att_src2,
               att_dst2, b2):
    src_o = np.concatenate([np.asarray(edge_index[0]),
                            np.arange(N, dtype=np.int64)]).astype(np.int64)
    dst_o = np.concatenate([np.asarray(edge_index[1]),
                            np.arange(N, dtype=np.int64)]).astype(np.int64)

    core_of = dst_o // NPC
    deg = np.bincount(dst_o, minlength=N)

    gid = np.full(N, -1, np.int64)
    node_of_gid = np.full(NGLOB, -1, np.int64)
    for c in range(NCORES):
        nodes = np.arange(c * NPC, (c + 1) * NPC)
        d = deg[nodes]
        order = np.argsort(-d, kind='stable')
        blk_cnt = np.zeros(NB, np.int64)
        blk_load = np.zeros(NB, np.int64)
        assign = np.full(NPC, -1, np.int64)
        for i in order:
            b = int(np.argmin(blk_load + (blk_cnt >= BLK) * (1 << 40)))
            assign[i] = b
            blk_cnt[b] += 1
            blk_load[b] += d[i]
        slot_ctr = np.zeros(NB, np.int64)
        for i in range(NPC):
            b = assign[i]
            g = c * NLOC + b * BLK + slot_ctr[b]
            slot_ctr[b] += 1
            gid[nodes[i]] = g
            node_of_gid[g] = nodes[i]

    src_g = gid[src_o]
    dst_g = gid[dst_o]
    dst_block = (dst_g % NLOC) // BLK
    dst_slot = dst_g % BLK
    side = (src_g >= NHALF).astype(np.int64)

    idxL1 = np.full((NCORES, NB, 2, SIDE), -1, np.int32)
    dslL1 = np.full((NCORES, NB, 2, SIDE), -1, np.int32)
    idxL2 = np.full((NCORES, NB, SIDE2), -1, np.int32)
    dslL2 = np.full((NCORES, NB, SIDE2), -1, np.int32)
    qofL2 = np.zeros((NCORES, NB, SIDE2), np.int32)
    cntL1 = np.zeros((NCORES, NB, 2), np.int64)
    cntL2 = np.zeros((NCORES, NB), np.int64)

    for c in range(NCORES):
        em = np.nonzero(core_of == c)[0]
        eb = dst_block[em]
        for b in range(NB):
            eidx = em[eb == b]
            srcs = src_g[eidx]
            dslots = dst_slot[eidx]
            sides = side[eidx]
            for s in (0, 1):
                ms = sides == s
                k = int(ms.sum())
                if k > SIDE:
                    raise RuntimeError(f"L1 overflow c{c} b{b} s{s}: {k}")
                idxL1[c, b, s, :k] = srcs[ms] - s * NHALF
                dslL1[c, b, s, :k] = dslots[ms]
                cntL1[c, b, s] = k
            k = len(eidx)
            if k > SIDE2:
                raise RuntimeError(f"L2 overflow c{c} b{b}: {k}")
            idxL2[c, b, :k] = srcs // L2Q
            qofL2[c, b, :k] = srcs % L2Q
            dslL2[c, b, :k] = dslots
            cntL2[c, b] = k

    # SPMD: same valid-descriptor count on every core -> pad with idx 0
    nvL1 = cntL1.max(axis=0)      # [NB, 2] total per side
    nvL2t = cntL2.max(axis=0)     # [NB] total per block
    for c in range(NCORES):
        for b in range(NB):
            for s in (0, 1):
                k = int(cntL1[c, b, s])
                m = min(((int(nvL1[b, s]) + 63) // 64) * 64, SIDE)
                idxL1[c, b, s, k:m] = 0
            k = int(cntL2[c, b])
            m = min(((int(nvL2t[b]) + 63) // 64) * 64, SIDE2)
            idxL2[c, b, k:m] = 0

    def segsplit(total, lo, hi):
        v = int(np.clip(total - lo * 128, 0, (hi - lo) * 128))
        return min(((v + 63) // 64) * 64, (hi - lo) * 128)

    nvL1seg = np.zeros((NB, 2, len(L1SEG)), np.int64)
    nvL2seg = np.zeros((NB, len(L2SEG)), np.int64)
    for b in range(NB):
        for s in (0, 1):
            t = min(((int(nvL1[b, s]) + 63) // 64) * 64, SIDE)
            for gsi, (lo, hi) in enumerate(L1SEG):
                nvL1seg[b, s, gsi] = segsplit(t, lo, hi)
        t = min(((int(nvL2t[b]) + 63) // 64) * 64, SIDE2)
        for gsi, (lo, hi) in enumerate(L2SEG):
            nvL2seg[b, gsi] = segsplit(t, lo, hi)

    SL1 = np.zeros((NCORES, NB, 128, 2 * CPS * 128), ml_dtypes.bfloat16)
    STL1 = np.zeros((NCORES, NB, 128, 2 * CPS * 128), ml_dtypes.bfloat16)
    SL2 = np.zeros((NCORES, NB, 128, CH2 * 128), ml_dtypes.bfloat16)
    STL2 = np.zeros((NCORES, NB, 128, CH2 * 128), ml_dtypes.bfloat16)
    QSELa = np.zeros((NCORES, NB, 128, CH2 * L2Q), ml_dtypes.bfloat16)
    for c in range(NCORES):
        for b in range(NB):
            for s in (0, 1):
                for j in range(CPS):
                    ch = s * CPS + j
                    sl = dslL1[c, b, s, j * 128:(j + 1) * 128]
                    e_i = np.nonzero(sl >= 0)[0]
                    n_i = sl[e_i]
                    SL1[c, b, e_i, ch * 128 + n_i] = 1
                    STL1[c, b, n_i, ch * 128 + e_i] = 1
            for j in range(CH2):
                sl = dslL2[c, b, j * 128:(j + 1) * 128]
                e_i = np.nonzero(sl >= 0)[0]
                n_i = sl[e_i]
                SL2[c, b, e_i, j * 128 + n_i] = 1
                STL2[c, b, n_i, j * 128 + e_i] = 1
                q = qofL2[c, b, j * 128:(j + 1) * 128]
                QSELa[c, b, e_i, j * L2Q + q[e_i]] = 1

    W1 = np.asarray(W1, np.float32)
    att_src1 = np.asarray(att_src1, np.float32)
    att_dst1 = np.asarray(att_dst1, np.float32)
    b1 = np.asarray(b1, np.float32)
    W2 = np.asarray(W2, np.float32)
    att_src2 = np.asarray(att_src2, np.float32)
    att_dst2 = np.asarray(att_dst2, np.float32)
    b2 = np.asarray(b2, np.float32)

    Msrc = np.zeros((F_IN, HEADS), np.float32)
    Mdst = np.zeros((F_IN, HEADS), np.float32)
    for h in range(HEADS):
        Msrc[h * HID:(h + 1) * HID, h] = att_src1[h]
        Mdst[h * HID:(h + 1) * HID, h] = att_dst1[h]
    # h features stored channel-major: new index j <-> old (j%8)*64 + j//8,
    # so per-head broadcasts have innermost stride 1 (period 8).
    permH = np.array([(j % HEADS) * HID + j // HEADS for j in range(512)])
    W1aug = np.zeros((640, 528), np.float32)
    W1aug[:512, 0:512] = W1[:, permH]
    W1aug[:512, 512:520] = W1 @ Msrc
    W1aug[:512, 520:528] = W1 @ Mdst
    W1aug[512, 0:512] = b1[permH]
    W2aug = np.zeros((512, 69), np.float32)
    W2aug[:, 0:64] = W2[permH, :]
    W2aug[:, 64] = (W2 @ att_src2[0])[permH]
    W2aug[:, 65] = (W2 @ att_dst2[0])[permH]
    W2bias = np.zeros((128, 69), np.float32)
    W2bias[0, 0:64] = b2
    W2bias[0, 68] = 1.0

    x = np.asarray(x, np.float32)
    xTs = []
    for c in range(NCORES):
        cols = node_of_gid[c * NLOC:(c + 1) * NLOC]
        validc = cols >= 0
        xc = np.zeros((NLOC, F_IN), np.float32)
        xc[validc] = x[cols[validc]]
        xt = np.zeros((640, NLOC), np.float32)
        xt[:512] = xc.T
        xt[512] = 1.0
        xTs.append(xt.astype(ml_dtypes.bfloat16))

    Wc1 = SIDE // 16            # idx cols per L1 side
    Wc2 = SIDE2 // 16           # idx cols per L2 block
    idxL1_dev = np.zeros((NCORES, 128, NB * 2 * Wc1), np.int16)
    idxL2_dev = np.zeros((NCORES, 128, NB * Wc2), np.int16)
    for c in range(NCORES):
        for b in range(NB):
            for s in (0, 1):
                col = (b * 2 + s) * Wc1
                for (lo, hi) in L1SEG:
                    seg = idxL1[c, b, s, lo * 128:hi * 128]
                    w0 = lo * 8
                    idxL1_dev[c, :, col + w0:col + w0 + len(seg) // 16] = \
                        _wrap16(seg)
            col = b * Wc2
            for (lo, hi) in L2SEG:
                seg = idxL2[c, b, lo * 128:hi * 128]
                w0 = lo * 8
                idxL2_dev[c, :, col + w0:col + w0 + len(seg) // 16] = _wrap16(seg)

    in_maps = []
    W1aug_bf = W1aug.astype(ml_dtypes.bfloat16)
    W2aug_bf = W2aug.astype(ml_dtypes.bfloat16)
    W2bias_bf = W2bias.astype(ml_dtypes.bfloat16)
    for c in range(NCORES):
        in_maps.append({
            "xT": np.ascontiguousarray(xTs[c]),
            "W1aug": W1aug_bf,
            "W2aug": W2aug_bf,
            "W2bias": W2bias_bf,
            "idxL1": np.ascontiguousarray(idxL1_dev[c]),
            "idxL2": np.ascontiguousarray(idxL2_dev[c]),
            "SL1": np.ascontiguousarray(SL1[c]),
            "STL1": np.ascontiguousarray(STL1[c]),
            "SL2": np.ascontiguousarray(SL2[c]),
            "STL2": np.ascontiguousarray(STL2[c]),
            "QSEL": np.ascontiguousarray(QSELa[c]),
        })
    meta = {"node_of_gid": node_of_gid, "nvL1": nvL1seg, "nvL2": nvL2seg}
    return in_maps, meta


# ---------------- device program ----------------


def _build_program(nvL1, nvL2):
    _install_patches()
    nc = bass.Bass()
    AF = mybir.ActivationFunctionType
    OP = mybir.AluOpType

    xT = nc.dram_tensor("xT", [640, NLOC], bf16, kind="ExternalInput")
    W1a = nc.dram_tensor("W1aug", [640, 528], bf16, kind="ExternalInput")
    W2a = nc.dram_tensor("W2aug", [512, 69], bf16, kind="ExternalInput")
    W2b = nc.dram_tensor("W2bias", [128, 69], bf16, kind="ExternalInput")
    Wc1 = SIDE // 16
    Wc2 = SIDE2 // 16
    CH1 = 2 * CPS
    idxL1 = nc.dram_tensor("idxL1", [128, NB * 2 * Wc1], mybir.dt.int16,
                           kind="ExternalInput")
    idxL2 = nc.dram_tensor("idxL2", [128, NB * Wc2], mybir.dt.int16,
                           kind="ExternalInput")
    SL1 = nc.dram_tensor("SL1", [NB, 128, CH1 * 128], bf16, kind="ExternalInput")
    STL1 = nc.dram_tensor("STL1", [NB, 128, CH1 * 128], bf16, kind="ExternalInput")
    SL2 = nc.dram_tensor("SL2", [NB, 128, CH2 * 128], bf16, kind="ExternalInput")
    STL2 = nc.dram_tensor("STL2", [NB, 128, CH2 * 128], bf16, kind="ExternalInput")
    QSEL = nc.dram_tensor("QSEL", [NB, 128, CH2 * L2Q], bf16, kind="ExternalInput")

    out_lsm = nc.dram_tensor("out_lsm", [NLOC, 64], f32, kind="ExternalOutput")
    out_Z = nc.dram_tensor("out_Z", [NLOC, 64], f32, kind="ExternalOutput")

    Hcat_loc = nc.dram_tensor("Hcat_loc", [NLOC, HROW], bf16)
    Hcat_g = nc.dram_tensor("Hcat_g", [NGLOB, HROW], bf16, addr_space="Shared")
    Gcat_loc = nc.dram_tensor("Gcat_loc", [NLOC, GROW], bf16)
    Gcat_g = nc.dram_tensor("Gcat_g", [NGLOB, GROW], bf16, addr_space="Shared")

    groups = [list(range(NCORES))]

    with tile.TileContext(nc) as tc:
        with tc.tile_critical():
            nc.gpsimd.load_library(library_config.mlp)
        nvset = sorted({int(v) for v in nvL1.flatten()} |
                       {int(v) for v in nvL2.flatten()})
        nvregs = {}
        for v in nvset:
            if v > 0:
                nvregs[v] = nc.gpsimd.to_reg(v)

        with tc.tile_pool(name="const", bufs=1) as constp:
            w1t = constp.tile([128, 5 * 528], bf16)
            for k in range(5):
                nc.sync.dma_start(w1t[:, k * 528:(k + 1) * 528],
                                  W1a[k * 128:(k + 1) * 128, :])
            w2t = constp.tile([128, 4 * 69], bf16)
            for k in range(4):
                nc.sync.dma_start(w2t[:, k * 69:(k + 1) * 69],
                                  W2a[k * 128:(k + 1) * 128, :])
            w2bt = constp.tile([128, 69], bf16)
            nc.sync.dma_start(w2bt[:], W2b[:, :])
            e0ones = constp.tile([128, 128], bf16)
            nc.vector.memset(e0ones[:], 0.0)
            nc.vector.memset(e0ones[0:1, :], 1.0)
            ident = constp.tile([128, 128], bf16)
            from concourse.masks import make_identity
            make_identity(nc, ident[:])
            dt_all = constp.tile([128, NB * 16], bf16)
            dt2_all = constp.tile([128, NB * 2], bf16)
            idx1t = constp.tile([128, NB * 2 * Wc1], mybir.dt.int16)
            nc.sync.dma_start(idx1t[:], idxL1[:, :])
            idx2t = constp.tile([128, NB * Wc2], mybir.dt.int16)
            nc.sync.dma_start(idx2t[:], idxL2[:, :])

            # ---- phase 1: GEMM1 + Hcat rows ----
            with tc.tile_pool(name="p1", bufs=3) as p1, \
                 tc.tile_pool(name="ps1a", bufs=2, space="PSUM") as ps1a, \
                 tc.tile_pool(name="ps1b", bufs=2, space="PSUM") as ps1b:
                for b in range(NB):
                    pA = ps1a.tile([128, 512], f32)
                    pB = ps1b.tile([128, 16], f32)
                    for k in range(5):
                        xt = p1.tile([128, 128], bf16, tag="xt")
                        nc.sync.dma_start(
                            xt[:], xT[k * 128:(k + 1) * 128, b * 128:(b + 1) * 128])
                        nc.tensor.matmul(pA[:], xt[:], w1t[:, k * 528:k * 528 + 512],
                                         start=(k == 0), stop=(k == 4))
                        nc.tensor.matmul(pB[:], xt[:],
                                         w1t[:, k * 528 + 512:(k + 1) * 528],
                                         start=(k == 0), stop=(k == 4))
                    hc = p1.tile([128, HROW], bf16, tag="hc")
                    nc.vector.tensor_copy(hc[:, 0:512], pA[:])
                    nc.scalar.activation(hc[:, 512:520], pB[:, 0:8], AF.Exp)
                    nc.scalar.activation(hc[:, 520:528], pB[:, 0:8], AF.Exp,
                                         scale=NEG_SLOPE)
                    nc.scalar.activation(hc[:, 528:536], pB[:, 8:16], AF.Exp)
                    nc.scalar.activation(hc[:, 536:544], pB[:, 8:16], AF.Exp,
                                         scale=NEG_SLOPE)
                    nc.vector.memset(hc[:, 544:HROW], 0.0)
                    nc.vector.tensor_copy(dt_all[:, b * 16:(b + 1) * 16],
                                          hc[:, 528:544])
                    nc.sync.dma_start(Hcat_loc[b * 128:(b + 1) * 128, :], hc[:])

            # ---- phase 2: AllGather Hcat ----
            nc.gpsimd.collective_compute(
                "AllGather", mybir.AluOpType.bypass, replica_groups=groups,
                ins=[Hcat_loc[:, :]], outs=[Hcat_g[:, :]])

            # ---- phase 3: L1 edges + block tails + GEMM2 + Gcat ----
            with tc.tile_pool(name="p3", bufs=2) as p3, \
                 tc.tile_pool(name="pg3", bufs=1) as pg3, \
                 tc.tile_pool(name="p3s", bufs=2) as p3s, \
                 tc.tile_pool(name="psA", bufs=2, space="PSUM") as psA, \
                 tc.tile_pool(name="psB", bufs=2, space="PSUM") as psB, \
                 tc.tile_pool(name="psuv", bufs=1, space="PSUM") as psuv, \
                 tc.tile_pool(name="pst", bufs=1, space="PSUM") as pst, \
                 tc.tile_pool(name="ps2", bufs=1, space="PSUM") as ps2:
                for b in range(NB):
                    sb = p3s.tile([128, CH1 * 128], bf16, tag="sb")
                    nc.sync.dma_start(sb[:], SL1[b, :, :])
                    stb = p3s.tile([128, CH1 * 128], bf16, tag="stb")
                    nc.sync.dma_start(stb[:], STL1[b, :, :])
                    gl = []
                    for s in (0, 1):
                        g = pg3.tile([128, CPS * HROW], bf16, tag=f"g{s}{b % 2}")
                        gv = g[:].rearrange("p (c w) -> p c w", w=HROW)
                        if b < 2:
                            nc.vector.memset(g[:], 0.0)
                        icol = (b * 2 + s) * Wc1
                        for gsi, (lo, hi) in enumerate(L1SEG):
                            nvs = int(nvL1[b, s, gsi])
                            if nvs == 0:
                                continue
                            nc.gpsimd.dma_gather(
                                out_ap=gv[:, lo:hi, :],
                                in_ap=Hcat_g[s * NHALF:(s + 1) * NHALF, :],
                                idxs_ap=idx1t[:, icol + lo * 8:icol + hi * 8],
                                num_idxs=(hi - lo) * 128, num_idxs_reg=nvregs[nvs],
                                elem_size=HROW)
                        gl.append(g)
                    puv = psuv.tile([128, CH1 * 16], f32)
                    for c in range(CH1):
                        nc.tensor.matmul(puv[:, c * 16:(c + 1) * 16],
                                         stb[:, c * 128:(c + 1) * 128],
                                         dt_all[:, b * 16:(b + 1) * 16],
                                         start=True, stop=True)
                    uvb = p3.tile([128, CH1 * 16], bf16, tag="uvb")
                    nc.scalar.activation(uvb[:], puv[:], AF.Copy)
                    pA = psA.tile([128, 512], f32)
                    pB = psB.tile([128, 8], f32)
                    hs = p3.tile([128, CH1 * 520], bf16, tag="hs")
                    hs3 = hs[:].rearrange("p (c w) -> p c w", w=520)
                    exw = p3.tile([128, CH1 * 512], bf16, tag="exw")
                    uvs = uvb[:].rearrange("p (c w) -> p c w", w=16)
                    for s in (0, 1):
                        g3 = gl[s][:].rearrange("p (c w) -> p c w", w=HROW)
                        tU = p3.tile([128, CPS * 8], f32, tag="tU")
                        tV = p3.tile([128, CPS * 8], f32, tag="tV")
                        exb = p3.tile([128, CPS * 8], bf16, tag="exb")
                        nc.vector.tensor_tensor(
                            out=tU[:].rearrange("p (c w) -> p c w", w=8),
                            in0=g3[:, :, 512:520],
                            in1=uvs[:, s * CPS:(s + 1) * CPS, 0:8],
                            op=OP.mult)
                        nc.vector.tensor_tensor(
                            out=tV[:].rearrange("p (c w) -> p c w", w=8),
                            in0=g3[:, :, 520:528],
                            in1=uvs[:, s * CPS:(s + 1) * CPS, 8:16],
                            op=OP.mult)
                        nc.vector.tensor_tensor(out=exb[:], in0=tU[:], in1=tV[:],
                                                op=OP.max)
                        for j in range(CPS):
                            c = s * CPS + j
                            nc.sync.dma_start(
                                exw[:, c * 512:(c + 1) * 512].rearrange(
                                    "p (a w) -> p a w", a=64),
                                exb[:, j * 8:(j + 1) * 8].rearrange(
                                    "p (o w) -> p o w", o=1).to_broadcast(
                                        [128, 64, 8]))
                        nc.vector.tensor_tensor(
                            out=hs3[:, s * CPS:(s + 1) * CPS, 0:512],
                            in0=g3[:, :, 0:512],
                            in1=exw[:].rearrange("p (c w) -> p c w", w=512)[
                                :, s * CPS:(s + 1) * CPS, :],
                            op=OP.mult)
                        nc.vector.tensor_copy(
                            hs3[:, s * CPS:(s + 1) * CPS, 512:520],
                            exb[:].rearrange("p (c w) -> p c w", w=8))
                    for c in range(CH1):
                        nc.tensor.matmul(pA[:], sb[:, c * 128:(c + 1) * 128],
                                         hs3[:, c, 0:512],
                                         start=(c == 0), stop=(c == CH1 - 1))
                        nc.tensor.matmul(pB[:], sb[:, c * 128:(c + 1) * 128],
                                         hs3[:, c, 512:520],
                                         start=(c == 0), stop=(c == CH1 - 1))
                    sden = p3.tile([128, 8], f32, tag="sden")
                    nc.vector.tensor_scalar_add(sden[:], pB[:], EPS)
                    rec = p3.tile([128, 8], f32, tag="rec")
                    nc.vector.reciprocal(rec[:], sden[:])
                    recw = p3.tile([128, 512], f32, tag="recw")
                    nc.sync.dma_start(
                        recw[:].rearrange("p (a w) -> p a w", a=64),
                        rec[:].rearrange("p (o w) -> p o w", o=1).to_broadcast(
                            [128, 64, 8]))
                    o1 = p3.tile([128, 512], f32, tag="o1")
                    nc.vector.tensor_tensor(out=o1[:], in0=pA[:], in1=recw[:],
                                            op=OP.mult)
                    rneg = p3.tile([128, 512], f32, tag="rneg")
                    nc.scalar.activation(rneg[:], o1[:], AF.Relu, scale=-1.0)
                    eneg = p3.tile([128, 512], f32, tag="eneg")
                    nc.scalar.activation(eneg[:], rneg[:], AF.Exp, scale=-1.0)
                    x1 = p3.tile([128, 512], f32, tag="x1")
                    nc.vector.tensor_scalar_add(x1[:], o1[:], 1.0)
                    m1 = p3.tile([128, 512], f32, tag="m1")
                    nc.vector.tensor_tensor(out=m1[:], in0=x1[:], in1=eneg[:],
                                            op=OP.max)
                    h1b = p3.tile([128, 512], bf16, tag="h1b")
                    nc.vector.tensor_scalar_add(h1b[:], m1[:], -1.0)
                    p2t = ps2.tile([128, 69], f32)
                    for k in range(4):
                        ptt = pst.tile([128, 128], bf16)
                        nc.tensor.transpose(ptt[:], h1b[:, k * 128:(k + 1) * 128],
                                            ident[:])
                        h1tk = p3.tile([128, 128], bf16, tag="h1tk")
                        nc.scalar.activation(h1tk[:], ptt[:], AF.Copy)
                        nc.tensor.matmul(p2t[:], h1tk[:],
                                         w2t[:, k * 69:(k + 1) * 69],
                                         start=(k == 0), stop=False)
                    nc.tensor.matmul(p2t[:], e0ones[:], w2bt[:],
                                     start=False, stop=True)
                    gt = p3.tile([128, GROW], bf16, tag="gt")
                    nc.scalar.activation(gt[:, 0:64], p2t[:, 0:64], AF.Copy)
                    nc.scalar.activation(gt[:, 64:65], p2t[:, 64:65], AF.Exp)
                    nc.scalar.activation(gt[:, 65:66], p2t[:, 64:65], AF.Exp,
                                         scale=NEG_SLOPE)
                    nc.scalar.activation(gt[:, 66:67], p2t[:, 65:66], AF.Exp)
                    nc.scalar.activation(gt[:, 67:68], p2t[:, 65:66], AF.Exp,
                                         scale=NEG_SLOPE)
                    nc.scalar.activation(gt[:, 68:69], p2t[:, 68:69], AF.Copy)
                    nc.vector.memset(gt[:, 69:GROW], 0.0)
                    nc.vector.tensor_copy(dt2_all[:, b * 2:(b + 1) * 2],
                                          gt[:, 66:68])
                    nc.sync.dma_start(Gcat_loc[b * 128:(b + 1) * 128, :], gt[:])

            # ---- phase 4: AllGather Gcat ----
            nc.gpsimd.collective_compute(
                "AllGather", mybir.AluOpType.bypass, replica_groups=groups,
                ins=[Gcat_loc[:, :]], outs=[Gcat_g[:, :]])

            # ---- phase 5: L2 edges + log_softmax ----
            Gquad = Gcat_g[:, :].rearrange("(a b) w -> a (b w)", b=L2Q)
            with tc.tile_pool(name="p5", bufs=2) as p5, \
                 tc.tile_pool(name="pg5", bufs=1) as pg5, \
                 tc.tile_pool(name="p5s", bufs=2) as p5s, \
                 tc.tile_pool(name="psuv2", bufs=1, space="PSUM") as psuv2, \
                 tc.tile_pool(name="ps2b", bufs=2, space="PSUM") as ps2b:
                for b in range(NB):
                    s2 = p5s.tile([128, CH2 * 128], bf16, tag="s2")
                    nc.sync.dma_start(s2[:], SL2[b, :, :])
                    st2 = p5s.tile([128, CH2 * 128], bf16, tag="st2")
                    nc.sync.dma_start(st2[:], STL2[b, :, :])
                    qs = p5s.tile([128, CH2 * L2Q], bf16, tag="qs")
                    nc.sync.dma_start(qs[:], QSEL[b, :, :])
                    g4 = pg5.tile([128, CH2 * L2Q * GROW], bf16,
                                  tag=f"g4{b % 2}")
                    g4r = g4[:].rearrange("p (c w) -> p c w", w=L2Q * GROW)
                    if b < 2:
                        nc.vector.memset(g4[:], 0.0)
                    icol = b * Wc2
                    for gsi, (lo, hi) in enumerate(L2SEG):
                        nvs = int(nvL2[b, gsi])
                        if nvs == 0:
                            continue
                        nc.gpsimd.dma_gather(
                            out_ap=g4r[:, lo:hi, :],
                            in_ap=Gquad,
                            idxs_ap=idx2t[:, icol + lo * 8:icol + hi * 8],
                            num_idxs=(hi - lo) * 128, num_idxs_reg=nvregs[nvs],
                            elem_size=L2Q * GROW)
                    g4v = g4[:].rearrange("p (c q w) -> p c q w", q=L2Q, w=GROW)
                    puv2 = psuv2.tile([128, CH2 * 2], f32)
                    for j in range(CH2):
                        nc.tensor.matmul(puv2[:, j * 2:(j + 1) * 2],
                                         st2[:, j * 128:(j + 1) * 128],
                                         dt2_all[:, b * 2:(b + 1) * 2],
                                         start=True, stop=True)
                    uv2b = p5.tile([128, CH2 * 2], f32, tag="uv2b")
                    nc.scalar.activation(uv2b[:], puv2[:], AF.Copy)
                    uv2v = uv2b[:].rearrange("p (c w) -> p c w", w=2)
                    qs3 = qs[:].rearrange("p (c q) -> p c q", q=L2Q)
                    qs4 = qs[:].rearrange("p (c q o) -> p c q o", q=L2Q, o=1)
                    u2s = p5.tile([128, CH2 * L2Q], f32, tag="u2s")
                    v2s = p5.tile([128, CH2 * L2Q], f32, tag="v2s")
                    nc.vector.tensor_tensor(
                        out=u2s[:].rearrange("p (c q o) -> p c q o", q=L2Q, o=1),
                        in0=g4v[:, :, :, 64:65],
                        in1=qs4, op=OP.mult)
                    nc.vector.tensor_tensor(
                        out=v2s[:].rearrange("p (c q o) -> p c q o", q=L2Q, o=1),
                        in0=g4v[:, :, :, 65:66],
                        in1=qs4, op=OP.mult)
                    u2r = p5.tile([128, CH2], f32, tag="u2r")
                    v2r = p5.tile([128, CH2], f32, tag="v2r")
                    nc.vector.reduce_sum(
                        u2r[:].rearrange("p (c o) -> p c o", o=1),
                        u2s[:].rearrange("p (c q) -> p c q", q=L2Q),
                        axis=mybir.AxisListType.X)
                    nc.vector.reduce_sum(
                        v2r[:].rearrange("p (c o) -> p c o", o=1),
                        v2s[:].rearrange("p (c q) -> p c q", q=L2Q),
                        axis=mybir.AxisListType.X)
                    U2 = p5.tile([128, CH2], f32, tag="U2")
                    V2 = p5.tile([128, CH2], f32, tag="V2")
                    nc.vector.tensor_tensor(
                        out=U2[:].rearrange("p (c o) -> p c o", o=1),
                        in0=u2r[:].rearrange("p (c o) -> p c o", o=1),
                        in1=uv2v[:, :, 0:1], op=OP.mult)
                    nc.vector.tensor_tensor(
                        out=V2[:].rearrange("p (c o) -> p c o", o=1),
                        in0=v2r[:].rearrange("p (c o) -> p c o", o=1),
                        in1=uv2v[:, :, 1:2], op=OP.mult)
                    ex2 = p5.tile([128, CH2], bf16, tag="ex2")
                    nc.vector.tensor_tensor(out=ex2[:], in0=U2[:], in1=V2[:],
                                            op=OP.max)
                    exsel = p5.tile([128, CH2 * L2Q], f32, tag="exsel")
                    nc.vector.tensor_tensor(
                        out=exsel[:].rearrange("p (c q) -> p c q", q=L2Q),
                        in0=qs3,
                        in1=ex2[:].rearrange("p (c o) -> p c o", o=1).to_broadcast(
                            [128, CH2, L2Q]),
                        op=OP.mult)
                    p2b = ps2b.tile([128, 69], f32)
                    for j in range(CH2):
                        for q in range(L2Q):
                            sx = p5.tile([128, 128], bf16, tag="sx")
                            nc.vector.tensor_scalar_mul(
                                sx[:], s2[:, j * 128:(j + 1) * 128],
                                exsel[:, j * L2Q + q:j * L2Q + q + 1])
                            nc.tensor.matmul(
                                p2b[:], sx[:],
                                g4[:, (j * L2Q + q) * GROW:(j * L2Q + q) * GROW + 69],
                                start=(j == 0 and q == 0),
                                stop=(j == CH2 - 1 and q == L2Q - 1))
                    den2 = p5.tile([128, 1], f32, tag="den2")
                    nc.vector.tensor_scalar_add(den2[:], p2b[:, 68:69], EPS)
                    rec2 = p5.tile([128, 1], f32, tag="rec2")
                    nc.vector.reciprocal(rec2[:], den2[:])
                    Zt = p5.tile([128, 64], f32, tag="Zt")
                    nc.vector.tensor_scalar_mul(Zt[:], p2b[:, 0:64], rec2[:, 0:1])
                    nc.sync.dma_start(out_Z[b * 128:(b + 1) * 128, :], Zt[:])
                    mx = p5.tile([128, 1], f32, tag="mx")
                    nc.vector.reduce_max(mx[:], Zt[:], axis=mybir.AxisListType.X)
                    nmx = p5.tile([128, 1], f32, tag="nmx")
                    nc.vector.tensor_scalar_mul(nmx[:], mx[:], -1.0)
                    ez = p5.tile([128, 64], f32, tag="ez")
                    nc.scalar.activation(ez[:], Zt[:], AF.Exp, bias=nmx[:, 0:1])
                    sz = p5.tile([128, 1], f32, tag="sz")
                    nc.vector.reduce_sum(sz[:], ez[:], axis=mybir.AxisListType.X)
                    lse = p5.tile([128, 1], f32, tag="lse")
                    nc.scalar.activation(lse[:], sz[:], AF.Ln)
                    ot = p5.tile([128, 64], f32, tag="ot")
                    nc.vector.tensor_scalar(
                        out=ot[:], in0=Zt[:], scalar1=mx[:, 0:1],
                        scalar2=lse[:, 0:1], op0=OP.subtract, op1=OP.subtract)
                    nc.sync.dma_start(out_lsm[b * 128:(b + 1) * 128, :], ot[:])

    mybir.codegen_inst_isa_subclasses(nc)
    return nc


# ---------------- top-level entry ----------------

_CACHE = {}


def kernel(x, edge_index, W1, att_src1, att_dst1, b1, W2, att_src2, att_dst2,
           b2, _trace=False):
    in_maps, meta = _host_prep(x, edge_index, W1, att_src1, att_dst1, b1, W2,
                               att_src2, att_dst2, b2)
    if "prog" not in _CACHE:
        _CACHE["prog"] = _build_program(meta["nvL1"], meta["nvL2"])
    nc = _CACHE["prog"]
    res = run_bass_kernel_spmd(nc, in_maps, list(range(NCORES)), trace=_trace)
    node_of_gid = meta["node_of_gid"]
    out = np.zeros((N, 64), np.float32)
    Z = np.zeros((N, 64), np.float32)
    for c in range(NCORES):
        lsm = res.results[c]["out_lsm"]
        zz = res.results[c]["out_Z"]
        cols = node_of_gid[c * NLOC:(c + 1) * NLOC]
        valid = cols >= 0
        out[cols[valid]] = np.asarray(lsm)[valid]
        Z[cols[valid]] = np.asarray(zz)[valid]
    kernel._last_exec_ns = res.exec_time_ns
    kernel._last_res = res
    return (out, Z)



# revision 3
# speedup vs baseline: 2.8866x; 2.8866x over previous
"""GAT 2-layer kernel for 8 trn2 NeuronCores (self-contained).

Sharding: destination-node blocks across 8 cores. Per core: 49 blocks x 128
nodes. Layer GEMMs computed on owned nodes; per-node attention factors
(u=exp(a_src), v=exp(0.2 a_src), and dst-side) folded into gatherable row
tables (Hcat/Gcat) that are AllGathered; edge phases gather rows by src via
dma_gather and do segment-softmax-aggregation with per-chunk selection-matrix
matmuls (exp(leaky(x)) == max(exp(x), exp(0.2 x)) makes the logits separable).
Edge slots are split by source half (int16 gather indices); both layers share
the same slot assignment so one selection-matrix pair and one index table
serve both edge phases.
"""
import sys, types
sys.path.insert(0, '/opt/trn_rl_repo')

import numpy as np
import ml_dtypes

# ---------------- problem constants (hardcoded) ----------------
N = 50000
F_IN = 512
HID = 64
HEADS = 8
CLS = 64
NEG_SLOPE = 0.2
NCORES = 8
NPC = 6250
NB = 49
BLK = 128
NLOC = NB * BLK           # 6272
NGLOB = NCORES * NLOC     # 50176
CPS = 6                   # chunks per side per block
SIDE = CPS * 128          # 768 edge slots per side
CH1 = 2 * CPS             # 12 chunks per block
WCI = SIDE // 16          # 48 idx cols per side
HROW = 640                # Hcat row (bf16)
GROW = 128                # Gcat row (bf16)
NHALF = NGLOB // 2        # 25088
EPS = 1e-16


def _install_ntff_hook():
    if 'antenv.axon_hooks' in sys.modules:
        return
    try:
        sys.path.insert(0, '/root/.axon_site')
        from trn_agent_boot.trn_boot import _ntff_profile_via_ctypes
        hook = _ntff_profile_via_ctypes('/opt/axon/libaxon_pjrt.so')
    except Exception:
        return
    mod = types.ModuleType('antenv.axon_hooks')
    mod._hook = hook
    mod.get_axon_ntff_profile_hook = lambda: mod._hook
    mod.set_axon_ntff_profile_hook = lambda h: setattr(mod, '_hook', h)
    sys.modules['antenv.axon_hooks'] = mod


_install_ntff_hook()

import concourse.bass as bass
import concourse.mybir as mybir
import concourse.tile as tile
from concourse import library_config
from concourse.bass_utils import run_bass_kernel_spmd
from concourse.vector_clock import VectorClock, ScopedClock

bf16 = mybir.dt.bfloat16
f32 = mybir.dt.float32

# ------------- tile framework patches (walrus: 1 sync wait / inst) ---------


def _drain_and_barrier_split(self, tick_clock, wait_clock):
    nc = self.nc
    full = tick_clock.global_clock
    procs = [p for p in range(27) if full[p] > 0]
    for p in procs:
        sub = VectorClock([full[q] if q == p else 0 for q in range(27)])
        drain_inst = nc.sync.drain(fusable=False)
        wait_clock.add_sem_waits(drain_inst.ins, ScopedClock({None: sub}))
    if not procs:
        nc.sync.drain(fusable=False)
    nc.all_engine_barrier()
    assert self.sems is not None
    popped = nc._tile_sem_poison_stack.pop()
    assert popped is self._sem_poison
    nc.clear_and_free_semaphores(list(self.sems.allocated().values()))
    nc.all_engine_barrier()


def _split_excess_waits(nc):
    for bb in nc.main_func.blocks:
        insts = bb.instructions
        i = 0
        while i < len(insts):
            ins = insts[i]
            si = ins.sync_info
            if si is None:
                i += 1
                continue
            waits = list(si.on_wait)
            if len(waits) <= 1:
                i += 1
                continue
            keep, surplus = waits[:1], waits[1:]
            ins.sync_info = mybir.SyncInfo(on_wait=keep, on_update=list(si.on_update))
            nops = []
            for w in surplus:
                nop = mybir.InstNoOp(name=nc.get_next_instruction_name())
                nop.engine = ins.engine
                nop.sync_info = mybir.SyncInfo(on_wait=[w], on_update=[])
                nc.register_instruction(nop)
                nops.append(nop)
            for k, nop in enumerate(nops):
                insts.insert(i + k, nop)
            i += 1 + len(nops)


_PATCHED = False


def _install_patches():
    global _PATCHED
    if _PATCHED:
        return
    _orig_exit = tile.TileContext.__exit__

    def _exit_with_split(self, *a):
        r = _orig_exit(self, *a)
        _split_excess_waits(self.nc)
        return r

    tile.TileContext._drain_and_barrier = _drain_and_barrier_split
    tile.TileContext.__exit__ = _exit_with_split
    _PATCHED = True


# ---------------- host-side graph preprocessing ----------------


def _wrap16(flat_idx):
    W = len(flat_idx) // 16
    t = np.asarray(flat_idx, np.int16).reshape(W, 16).T
    return np.tile(t, (8, 1))


def _host_prep(x, edge_index, W1, att_src1, att_dst1, b1, W2, att_src2,
               att_dst2, b2):
    src_o = np.concatenate([np.asarray(edge_index[0]),
                            np.arange(N, dtype=np.int64)]).astype(np.int64)
    dst_o = np.concatenate([np.asarray(edge_index[1]),
                            np.arange(N, dtype=np.int64)]).astype(np.int64)

    core_of = dst_o // NPC
    deg = np.bincount(dst_o, minlength=N)

    gid = np.full(N, -1, np.int64)
    node_of_gid = np.full(NGLOB, -1, np.int64)
    for c in range(NCORES):
        nodes = np.arange(c * NPC, (c + 1) * NPC)
        d = deg[nodes]
        order = np.argsort(-d, kind='stable')
        blk_cnt = np.zeros(NB, np.int64)
        blk_load = np.zeros(NB, np.int64)
        assign = np.full(NPC, -1, np.int64)
        for i in order:
            b = int(np.argmin(blk_load + (blk_cnt >= BLK) * (1 << 40)))
            assign[i] = b
            blk_cnt[b] += 1
            blk_load[b] += d[i]
        slot_ctr = np.zeros(NB, np.int64)
        for i in range(NPC):
            b = assign[i]
            g = c * NLOC + b * BLK + slot_ctr[b]
            slot_ctr[b] += 1
            gid[nodes[i]] = g
            node_of_gid[g] = nodes[i]

    src_g = gid[src_o]
    dst_g = gid[dst_o]
    dst_block = (dst_g % NLOC) // BLK
    dst_slot = dst_g % BLK
    side = (src_g >= NHALF).astype(np.int64)

    idxL1 = np.full((NCORES, NB, 2, SIDE), -1, np.int32)
    dslL1 = np.full((NCORES, NB, 2, SIDE), -1, np.int32)
    cntL1 = np.zeros((NCORES, NB, 2), np.int64)

    for c in range(NCORES):
        em = np.nonzero(core_of == c)[0]
        eb = dst_block[em]
        for b in range(NB):
            eidx = em[eb == b]
            srcs = src_g[eidx]
            dslots = dst_slot[eidx]
            sides = side[eidx]
            for s in (0, 1):
                ms = sides == s
                k = int(ms.sum())
                if k > SIDE:
                    raise RuntimeError(f"L1 overflow c{c} b{b} s{s}: {k}")
                idxL1[c, b, s, :k] = srcs[ms] - s * NHALF
                dslL1[c, b, s, :k] = dslots[ms]
                cntL1[c, b, s] = k

    # SPMD: same valid-descriptor count on every core -> pad with idx 0
    nvmax = cntL1.max(axis=0)      # [NB, 2]
    nvL1 = np.minimum(((nvmax + 63) // 64) * 64, SIDE)
    for c in range(NCORES):
        for b in range(NB):
            for s in (0, 1):
                k = int(cntL1[c, b, s])
                idxL1[c, b, s, k:int(nvL1[b, s])] = 0

    SL1 = np.zeros((NCORES, NB, 128, CH1 * 128), ml_dtypes.bfloat16)
    STL1 = np.zeros((NCORES, NB, 128, CH1 * 128), ml_dtypes.bfloat16)
    for c in range(NCORES):
        for b in range(NB):
            for s in (0, 1):
                for j in range(CPS):
                    ch = s * CPS + j
                    sl = dslL1[c, b, s, j * 128:(j + 1) * 128]
                    e_i = np.nonzero(sl >= 0)[0]
                    n_i = sl[e_i]
                    SL1[c, b, e_i, ch * 128 + n_i] = 1
                    STL1[c, b, n_i, ch * 128 + e_i] = 1
    SB1 = np.concatenate([SL1, STL1], axis=-1)  # [NCORES, NB, 128, 2*CH1*128]

    W1 = np.asarray(W1, np.float32)
    att_src1 = np.asarray(att_src1, np.float32)
    att_dst1 = np.asarray(att_dst1, np.float32)
    b1 = np.asarray(b1, np.float32)
    W2 = np.asarray(W2, np.float32)
    att_src2 = np.asarray(att_src2, np.float32)
    att_dst2 = np.asarray(att_dst2, np.float32)
    b2 = np.asarray(b2, np.float32)

    Msrc = np.zeros((F_IN, HEADS), np.float32)
    Mdst = np.zeros((F_IN, HEADS), np.float32)
    for h in range(HEADS):
        Msrc[h * HID:(h + 1) * HID, h] = att_src1[h]
        Mdst[h * HID:(h + 1) * HID, h] = att_dst1[h]
    # h features stored head-major (natural W1 layout: col = head*64 + ch)
    W1aug = np.zeros((F_IN + 128, F_IN + 16), np.float32)
    W1aug[:F_IN, 0:F_IN] = W1
    W1aug[:F_IN, F_IN:F_IN + 8] = W1 @ Msrc
    W1aug[:F_IN, F_IN + 8:F_IN + 16] = W1 @ Mdst
    W1aug[F_IN, 0:F_IN] = b1
    W2aug = np.zeros((F_IN, 69), np.float32)
    W2aug[:, 0:CLS] = W2
    W2aug[:, 64] = W2 @ att_src2[0]
    W2aug[:, 65] = W2 @ att_dst2[0]
    W2bias = np.zeros((128, 69), np.float32)
    W2bias[0, 0:CLS] = b2
    W2bias[0, 68] = 1.0

    x = np.asarray(x, np.float32)
    xTs = []
    for c in range(NCORES):
        cols = node_of_gid[c * NLOC:(c + 1) * NLOC]
        validc = cols >= 0
        xc = np.zeros((NLOC, F_IN), np.float32)
        xc[validc] = x[cols[validc]]
        xt = np.zeros((F_IN + 128, NLOC), np.float32)
        xt[:F_IN] = xc.T
        xt[F_IN] = 1.0
        xTs.append(xt.astype(ml_dtypes.bfloat16))

    idxL1_dev = np.zeros((NCORES, 128, NB * 2 * WCI), np.int16)
    for c in range(NCORES):
        for b in range(NB):
            for s in (0, 1):
                col = (b * 2 + s) * WCI
                idxL1_dev[c, :, col:col + WCI] = _wrap16(idxL1[c, b, s, :])

    in_maps = []
    W1aug_bf = W1aug.astype(ml_dtypes.bfloat16)
    W2aug_bf = W2aug.astype(ml_dtypes.bfloat16)
    W2bias_bf = W2bias.astype(ml_dtypes.bfloat16)
    for c in range(NCORES):
        in_maps.append({
            "xT": np.ascontiguousarray(xTs[c]),
            "W1aug": W1aug_bf,
            "W2aug": W2aug_bf,
            "W2bias": W2bias_bf,
            "idxL1": np.ascontiguousarray(idxL1_dev[c]),
            "SB1": np.ascontiguousarray(SB1[c]),
        })
    meta = {"node_of_gid": node_of_gid, "nvL1": nvL1}
    return in_maps, meta


# ---------------- device program ----------------


def _build_program(nvL1):
    _install_patches()
    nc = bass.Bass()
    AF = mybir.ActivationFunctionType
    OP = mybir.AluOpType
    KW = (F_IN + 128) // 128          # 5 k-chunks for GEMM1
    WROW = F_IN + 16                  # 528 W1aug cols

    xT = nc.dram_tensor("xT", [F_IN + 128, NLOC], bf16, kind="ExternalInput")
    W1a = nc.dram_tensor("W1aug", [F_IN + 128, WROW], bf16, kind="ExternalInput")
    W2a = nc.dram_tensor("W2aug", [F_IN, 69], bf16, kind="ExternalInput")
    W2b = nc.dram_tensor("W2bias", [128, 69], bf16, kind="ExternalInput")
    idxL1 = nc.dram_tensor("idxL1", [128, NB * 2 * WCI], mybir.dt.int16,
                           kind="ExternalInput")
    SB1 = nc.dram_tensor("SB1", [NB, 128, 2 * CH1 * 128], bf16,
                         kind="ExternalInput")

    out_cat = nc.dram_tensor("out_cat", [NLOC, 128], f32, kind="ExternalOutput")

    Hcat_loc = nc.dram_tensor("Hcat_loc", [NLOC, HROW], bf16)
    Hcat_g = nc.dram_tensor("Hcat_g", [NGLOB, HROW], bf16, addr_space="Shared")
    Gcat_loc = nc.dram_tensor("Gcat_loc", [NLOC, GROW], bf16)
    Gcat_g = nc.dram_tensor("Gcat_g", [NGLOB, GROW], bf16, addr_space="Shared")

    groups = [list(range(NCORES))]

    with tile.TileContext(nc) as tc:
        with tc.tile_critical():
            nc.gpsimd.load_library(library_config.mlp)
        nvset = sorted({int(v) for v in nvL1.flatten()})
        nvregs = {}
        for v in nvset:
            if v > 0:
                nvregs[v] = nc.gpsimd.to_reg(v)

        with tc.tile_pool(name="const", bufs=1) as constp:
            w1t = constp.tile([128, KW * WROW], bf16)
            for k in range(KW):
                nc.sync.dma_start(w1t[:, k * WROW:(k + 1) * WROW],
                                  W1a[k * 128:(k + 1) * 128, :])
            w2t = constp.tile([128, 4 * 69], bf16)
            for k in range(4):
                nc.sync.dma_start(w2t[:, k * 69:(k + 1) * 69],
                                  W2a[k * 128:(k + 1) * 128, :])
            w2bt = constp.tile([128, 69], bf16)
            nc.sync.dma_start(w2bt[:], W2b[:, :])
            e0ones = constp.tile([128, 128], bf16)
            nc.vector.memset(e0ones[:], 0.0)
            nc.vector.memset(e0ones[0:1, :], 1.0)
            ident = constp.tile([128, 128], bf16)
            from concourse.masks import make_identity
            make_identity(nc, ident[:])
            dt_all = constp.tile([128, NB * 16], bf16)
            dt2_all = constp.tile([128, NB * 2], bf16)
            idx1t = constp.tile([128, NB * 2 * WCI], mybir.dt.int16)
            nc.sync.dma_start(idx1t[:], idxL1[:, :])

            # ---- phase 1: GEMM1 + Hcat rows ----
            with tc.tile_pool(name="p1", bufs=3) as p1, \
                 tc.tile_pool(name="ps1a", bufs=2, space="PSUM") as ps1a, \
                 tc.tile_pool(name="ps1b", bufs=2, space="PSUM") as ps1b:
                for b in range(NB):
                    pA = ps1a.tile([128, F_IN], f32)
                    pB = ps1b.tile([128, 16], f32)
                    xt = p1.tile([128, KW * 128], bf16, tag="xt")
                    nc.sync.dma_start(
                        xt[:].rearrange("p (k j) -> p k j", k=KW),
                        xT[0:KW * 128, b * 128:(b + 1) * 128].rearrange(
                            "(k p) j -> p k j", k=KW))
                    for k in range(KW):
                        nc.tensor.matmul(pA[:], xt[:, k * 128:(k + 1) * 128],
                                         w1t[:, k * WROW:k * WROW + F_IN],
                                         start=(k == 0), stop=(k == KW - 1))
                        nc.tensor.matmul(pB[:], xt[:, k * 128:(k + 1) * 128],
                                         w1t[:, k * WROW + F_IN:(k + 1) * WROW],
                                         start=(k == 0), stop=(k == KW - 1))
                    hc = p1.tile([128, HROW], bf16, tag="hc")
                    nc.vector.tensor_copy(hc[:, 0:F_IN], pA[:])
                    nc.scalar.activation(hc[:, 512:520], pB[:, 0:8], AF.Exp)
                    nc.scalar.activation(hc[:, 520:528], pB[:, 0:8], AF.Exp,
                                         scale=NEG_SLOPE)
                    nc.scalar.activation(hc[:, 528:536], pB[:, 8:16], AF.Exp)
                    nc.scalar.activation(hc[:, 536:544], pB[:, 8:16], AF.Exp,
                                         scale=NEG_SLOPE)
                    if b < 3:
                        nc.vector.memset(hc[:, 544:HROW], 0.0)
                    nc.vector.tensor_copy(dt_all[:, b * 16:(b + 1) * 16],
                                          hc[:, 528:544])
                    nc.scalar.dma_start(Hcat_loc[b * 128:(b + 1) * 128, :], hc[:])

            # ---- phase 2: AllGather Hcat ----
            nc.gpsimd.collective_compute(
                "AllGather", mybir.AluOpType.bypass, replica_groups=groups,
                ins=[Hcat_loc[:, :]], outs=[Hcat_g[:, :]])

            # ---- phase 3: L1 edges + block tails + GEMM2 + Gcat ----
            with tc.tile_pool(name="p3", bufs=3) as p3, \
                 tc.tile_pool(name="pg3", bufs=1) as pg3, \
                 tc.tile_pool(name="p3s", bufs=2) as p3s, \
                 tc.tile_pool(name="psA", bufs=2, space="PSUM") as psA, \
                 tc.tile_pool(name="psB", bufs=2, space="PSUM") as psB, \
                 tc.tile_pool(name="psuv", bufs=1, space="PSUM") as psuv, \
                 tc.tile_pool(name="pst", bufs=1, space="PSUM") as pst, \
                 tc.tile_pool(name="ps2", bufs=1, space="PSUM") as ps2:
                for b in range(NB):
                    sbst = p3s.tile([128, 2 * CH1 * 128], bf16, tag="sbst")
                    nc.sync.dma_start(sbst[:], SB1[b, :, :])
                    sb = sbst[:, 0:CH1 * 128]
                    stb = sbst[:, CH1 * 128:2 * CH1 * 128]
                    gl = []
                    for s in (0, 1):
                        g = pg3.tile([128, CPS * HROW], bf16, tag=f"g{s}{b % 2}")
                        gv = g[:].rearrange("p (c w) -> p c w", w=HROW)
                        if b < 2:
                            nc.vector.memset(g[:], 0.0)
                        nvs = int(nvL1[b, s])
                        if nvs > 0:
                            nc.gpsimd.dma_gather(
                                out_ap=gv[:, :, :],
                                in_ap=Hcat_g[s * NHALF:(s + 1) * NHALF, :],
                                idxs_ap=idx1t[:, (b * 2 + s) * WCI:
                                              (b * 2 + s + 1) * WCI],
                                num_idxs=SIDE, num_idxs_reg=nvregs[nvs],
                                elem_size=HROW)
                        gl.append(g)
                    puv = psuv.tile([128, CH1 * 16], f32)
                    for c in range(CH1):
                        nc.tensor.matmul(puv[:, c * 16:(c + 1) * 16],
                                         stb[:, c * 128:(c + 1) * 128],
                                         dt_all[:, b * 16:(b + 1) * 16],
                                         start=True, stop=True)
                    uvb = p3.tile([128, CH1 * 16], bf16, tag="uvb")
                    nc.scalar.activation(uvb[:], puv[:], AF.Copy)
                    pA = psA.tile([128, F_IN], f32)
                    pB = psB.tile([128, 8], f32)
                    hs = p3.tile([128, CH1 * 520], bf16, tag="hs")
                    hs3 = hs[:].rearrange("p (c w) -> p c w", w=520)
                    uvs = uvb[:].rearrange("p (c w) -> p c w", w=16)
                    for s in (0, 1):
                        g3 = gl[s][:].rearrange("p (c w) -> p c w", w=HROW)
                        tU = p3.tile([128, CPS * 8], f32, tag="tU")
                        tV = p3.tile([128, CPS * 8], f32, tag="tV")
                        exb = p3.tile([128, CPS * 8], bf16, tag=f"exb{s}")
                        nc.vector.tensor_tensor(
                            out=tU[:].rearrange("p (c w) -> p c w", w=8),
                            in0=g3[:, :, 512:520],
                            in1=uvs[:, s * CPS:(s + 1) * CPS, 0:8],
                            op=OP.mult)
                        nc.vector.tensor_tensor(
                            out=tV[:].rearrange("p (c w) -> p c w", w=8),
                            in0=g3[:, :, 520:528],
                            in1=uvs[:, s * CPS:(s + 1) * CPS, 8:16],
                            op=OP.mult)
                        nc.vector.tensor_tensor(out=exb[:], in0=tU[:], in1=tV[:],
                                                op=OP.max)
                        nc.vector.tensor_tensor(
                            out=hs3[:, s * CPS:(s + 1) * CPS, 0:F_IN].rearrange(
                                "p c (h w) -> p c h w", h=HEADS),
                            in0=g3[:, :, 0:F_IN].rearrange(
                                "p c (h w) -> p c h w", h=HEADS),
                            in1=exb[:].rearrange("p (c h o) -> p c h o",
                                                 h=HEADS, o=1).to_broadcast(
                                [128, CPS, HEADS, HID]),
                            op=OP.mult)
                        nc.vector.tensor_copy(
                            hs3[:, s * CPS:(s + 1) * CPS, 512:520],
                            exb[:].rearrange("p (c w) -> p c w", w=8))
                    for c in range(CH1):
                        nc.tensor.matmul(pA[:], sb[:, c * 128:(c + 1) * 128],
                                         hs3[:, c, 0:F_IN],
                                         start=(c == 0), stop=(c == CH1 - 1))
                        nc.tensor.matmul(pB[:], sb[:, c * 128:(c + 1) * 128],
                                         hs3[:, c, 512:520],
                                         start=(c == 0), stop=(c == CH1 - 1))
                    sden = p3.tile([128, 8], f32, tag="sden")
                    nc.vector.tensor_scalar_add(sden[:], pB[:], EPS)
                    rec = p3.tile([128, 8], f32, tag="rec")
                    nc.vector.reciprocal(rec[:], sden[:])
                    o1 = p3.tile([128, F_IN], f32, tag="o1")
                    nc.vector.tensor_tensor(
                        out=o1[:].rearrange("p (h w) -> p h w", h=HEADS),
                        in0=pA[:].rearrange("p (h w) -> p h w", h=HEADS),
                        in1=rec[:].rearrange("p (h o) -> p h o", o=1).to_broadcast(
                            [128, HEADS, HID]),
                        op=OP.mult)
                    rneg = p3.tile([128, F_IN], f32, tag="rneg")
                    nc.scalar.activation(rneg[:], o1[:], AF.Relu, scale=-1.0)
                    eneg = p3.tile([128, F_IN], f32, tag="eneg")
                    nc.scalar.activation(eneg[:], rneg[:], AF.Exp, scale=-1.0)
                    x1 = p3.tile([128, F_IN], f32, tag="x1")
                    nc.vector.tensor_scalar_add(x1[:], o1[:], 1.0)
                    m1 = p3.tile([128, F_IN], f32, tag="m1")
                    nc.vector.tensor_tensor(out=m1[:], in0=x1[:], in1=eneg[:],
                                            op=OP.max)
                    h1b = p3.tile([128, F_IN], bf16, tag="h1b")
                    nc.vector.tensor_scalar_add(h1b[:], m1[:], -1.0)
                    p2t = ps2.tile([128, 69], f32)
                    for k in range(4):
                        ptt = pst.tile([128, 128], bf16)
                        nc.tensor.transpose(ptt[:], h1b[:, k * 128:(k + 1) * 128],
                                            ident[:])
                        h1tk = p3.tile([128, 128], bf16, tag="h1tk")
                        nc.scalar.activation(h1tk[:], ptt[:], AF.Copy)
                        nc.tensor.matmul(p2t[:], h1tk[:],
                                         w2t[:, k * 69:(k + 1) * 69],
                                         start=(k == 0), stop=False)
                    nc.tensor.matmul(p2t[:], e0ones[:], w2bt[:],
                                     start=False, stop=True)
                    gt = p3.tile([128, GROW], bf16, tag="gt")
                    nc.scalar.activation(gt[:, 0:64], p2t[:, 0:64], AF.Copy)
                    nc.scalar.activation(gt[:, 64:65], p2t[:, 64:65], AF.Exp)
                    nc.scalar.activation(gt[:, 65:66], p2t[:, 64:65], AF.Exp,
                                         scale=NEG_SLOPE)
                    nc.scalar.activation(gt[:, 66:67], p2t[:, 65:66], AF.Exp)
                    nc.scalar.activation(gt[:, 67:68], p2t[:, 65:66], AF.Exp,
                                         scale=NEG_SLOPE)
                    nc.scalar.activation(gt[:, 68:69], p2t[:, 68:69], AF.Copy)
                    if b < 3:
                        nc.vector.memset(gt[:, 69:GROW], 0.0)
                    nc.vector.tensor_copy(dt2_all[:, b * 2:(b + 1) * 2],
                                          gt[:, 66:68])
                    nc.scalar.dma_start(Gcat_loc[b * 128:(b + 1) * 128, :], gt[:])

            # ---- phase 4: AllGather Gcat ----
            nc.gpsimd.collective_compute(
                "AllGather", mybir.AluOpType.bypass, replica_groups=groups,
                ins=[Gcat_loc[:, :]], outs=[Gcat_g[:, :]])

            # ---- phase 5: L2 edges + log_softmax ----
            with tc.tile_pool(name="p5", bufs=3) as p5, \
                 tc.tile_pool(name="pg5", bufs=1) as pg5, \
                 tc.tile_pool(name="p5s", bufs=2) as p5s, \
                 tc.tile_pool(name="psuv2", bufs=1, space="PSUM") as psuv2, \
                 tc.tile_pool(name="ps2b", bufs=2, space="PSUM") as ps2b:
                for b in range(NB):
                    sbst5 = p5s.tile([128, 2 * CH1 * 128], bf16, tag="sbst5")
                    nc.sync.dma_start(sbst5[:], SB1[b, :, :])
                    sb5 = sbst5[:, 0:CH1 * 128]
                    st5 = sbst5[:, CH1 * 128:2 * CH1 * 128]
                    g4 = pg5.tile([128, CH1 * GROW], bf16, tag=f"g4{b % 2}")
                    g4r = g4[:].rearrange("p (c w) -> p c w", w=GROW)
                    if b < 2:
                        nc.vector.memset(g4[:], 0.0)
                    for s in (0, 1):
                        nvs = int(nvL1[b, s])
                        if nvs > 0:
                            nc.gpsimd.dma_gather(
                                out_ap=g4r[:, s * CPS:(s + 1) * CPS, :],
                                in_ap=Gcat_g[s * NHALF:(s + 1) * NHALF, :],
                                idxs_ap=idx1t[:, (b * 2 + s) * WCI:
                                              (b * 2 + s + 1) * WCI],
                                num_idxs=SIDE, num_idxs_reg=nvregs[nvs],
                                elem_size=GROW)
                    puv2 = psuv2.tile([128, CH1 * 2], f32)
                    for c in range(CH1):
                        nc.tensor.matmul(puv2[:, c * 2:(c + 1) * 2],
                                         st5[:, c * 128:(c + 1) * 128],
                                         dt2_all[:, b * 2:(b + 1) * 2],
                                         start=True, stop=True)
                    uv2b = p5.tile([128, CH1 * 2], bf16, tag="uv2b")
                    nc.scalar.activation(uv2b[:], puv2[:], AF.Copy)
                    uv2v = uv2b[:].rearrange("p (c w) -> p c w", w=2)
                    U2 = p5.tile([128, CH1], f32, tag="U2")
                    V2 = p5.tile([128, CH1], f32, tag="V2")
                    nc.vector.tensor_tensor(
                        out=U2[:].rearrange("p (c o) -> p c o", o=1),
                        in0=g4r[:, :, 64:65], in1=uv2v[:, :, 0:1], op=OP.mult)
                    nc.vector.tensor_tensor(
                        out=V2[:].rearrange("p (c o) -> p c o", o=1),
                        in0=g4r[:, :, 65:66], in1=uv2v[:, :, 1:2], op=OP.mult)
                    ex2 = p5.tile([128, CH1], bf16, tag="ex2")
                    nc.vector.tensor_tensor(out=ex2[:], in0=U2[:], in1=V2[:],
                                            op=OP.max)
                    g4f = p5.tile([128, CH1 * GROW], bf16, tag="g4f")
                    nc.vector.tensor_tensor(
                        out=g4f[:].rearrange("p (c w) -> p c w", w=GROW),
                        in0=g4r,
                        in1=ex2[:].rearrange("p (c o) -> p c o", o=1).to_broadcast(
                            [128, CH1, GROW]),
                        op=OP.mult)
                    g4f3 = g4f[:].rearrange("p (c w) -> p c w", w=GROW)
                    p2b = ps2b.tile([128, 69], f32)
                    for c in range(CH1):
                        nc.tensor.matmul(p2b[:], sb5[:, c * 128:(c + 1) * 128],
                                         g4f3[:, c, 0:69],
                                         start=(c == 0), stop=(c == CH1 - 1))
                    den2 = p5.tile([128, 1], f32, tag="den2")
                    nc.vector.tensor_scalar_add(den2[:], p2b[:, 68:69], EPS)
                    rec2 = p5.tile([128, 1], f32, tag="rec2")
                    nc.vector.reciprocal(rec2[:], den2[:])
                    ozt = p5.tile([128, 128], f32, tag="ozt")
                    Zt = ozt[:, 64:128]
                    nc.vector.tensor_scalar_mul(Zt, p2b[:, 0:64], rec2[:, 0:1])
                    mx = p5.tile([128, 1], f32, tag="mx")
                    nc.vector.reduce_max(mx[:], Zt, axis=mybir.AxisListType.X)
                    nmx = p5.tile([128, 1], f32, tag="nmx")
                    nc.vector.tensor_scalar_mul(nmx[:], mx[:], -1.0)
                    ez = p5.tile([128, 64], f32, tag="ez")
                    nc.scalar.activation(ez[:], Zt, AF.Exp, bias=nmx[:, 0:1])
                    sz = p5.tile([128, 1], f32, tag="sz")
                    nc.vector.reduce_sum(sz[:], ez[:], axis=mybir.AxisListType.X)
                    lse = p5.tile([128, 1], f32, tag="lse")
                    nc.scalar.activation(lse[:], sz[:], AF.Ln)
                    nc.vector.tensor_scalar(
                        out=ozt[:, 0:64], in0=Zt, scalar1=mx[:, 0:1],
                        scalar2=lse[:, 0:1], op0=OP.subtract, op1=OP.subtract)
                    nc.scalar.dma_start(out_cat[b * 128:(b + 1) * 128, :], ozt[:])

    mybir.codegen_inst_isa_subclasses(nc)
    return nc


# ---------------- top-level entry ----------------

_CACHE = {}


def kernel(x, edge_index, W1, att_src1, att_dst1, b1, W2, att_src2, att_dst2,
           b2, _trace=False):
    in_maps, meta = _host_prep(x, edge_index, W1, att_src1, att_dst1, b1, W2,
                               att_src2, att_dst2, b2)
    if "prog" not in _CACHE:
        _CACHE["prog"] = _build_program(meta["nvL1"])
    nc = _CACHE["prog"]
    res = run_bass_kernel_spmd(nc, in_maps, list(range(NCORES)), trace=_trace)
    node_of_gid = meta["node_of_gid"]
    out = np.zeros((N, 64), np.float32)
    Z = np.zeros((N, 64), np.float32)
    for c in range(NCORES):
        oc = np.asarray(res.results[c]["out_cat"])
        cols = node_of_gid[c * NLOC:(c + 1) * NLOC]
        valid = cols >= 0
        out[cols[valid]] = oc[valid, 0:64]
        Z[cols[valid]] = oc[valid, 64:128]
    kernel._last_exec_ns = res.exec_time_ns
    kernel._last_res = res
    return (out, Z)


# revision 9
# speedup vs baseline: 3.7052x; 1.2836x over previous
"""GAT 2-layer kernel for 8 trn2 NeuronCores (self-contained).

Sharding: destination-node blocks across 8 cores. Per core: 49 blocks x 128
nodes. Layer GEMMs computed on owned nodes; per-node attention factors
(u=exp(a_src), v=exp(0.2 a_src), and dst-side) folded into gatherable row
tables (Hcat/Gcat) that are AllGathered; edge phases gather rows by src via
dma_gather and do segment-softmax-aggregation with per-chunk selection-matrix
matmuls (exp(leaky(x)) == max(exp(x), exp(0.2 x)) makes the logits separable).
Edge slots are split by source half (int16 gather indices); both layers share
the same slot assignment so one selection-matrix pair and one index table
serve both edge phases.
"""
import sys, types
sys.path.insert(0, '/opt/trn_rl_repo')

import numpy as np
import ml_dtypes

# ---------------- problem constants (hardcoded) ----------------
N = 50000
F_IN = 512
HID = 64
HEADS = 8
CLS = 64
NEG_SLOPE = 0.2
NCORES = 8
NPC = 6250
NB = 49
BLK = 128
NLOC = NB * BLK           # 6272
NGLOB = NCORES * NLOC     # 50176
CPS = 6                   # chunks per side per block
SIDE = CPS * 128          # 768 edge slots per side
CH1 = 2 * CPS             # 12 chunks per block
WCI = SIDE // 16          # 48 idx cols per side
HROW = 640                # Hcat row (bf16)
GROW = 128                # Gcat row (bf16)
AGS0 = 25                 # blocks per core in AG chunk A
R0 = AGS0 * BLK           # 3200 rows per core in chunk A
R1 = NLOC - R0            # 3072 rows per core in chunk B
S0TOT = NCORES * R0       # 25600 rows in side/chunk 0
S1TOT = NCORES * R1       # 24576 rows in side/chunk 1
EPS = 1e-16


def _install_ntff_hook():
    if 'antenv.axon_hooks' in sys.modules:
        return
    try:
        sys.path.insert(0, '/root/.axon_site')
        from trn_agent_boot.trn_boot import _ntff_profile_via_ctypes
        hook = _ntff_profile_via_ctypes('/opt/axon/libaxon_pjrt.so')
    except Exception:
        return
    mod = types.ModuleType('antenv.axon_hooks')
    mod._hook = hook
    mod.get_axon_ntff_profile_hook = lambda: mod._hook
    mod.set_axon_ntff_profile_hook = lambda h: setattr(mod, '_hook', h)
    sys.modules['antenv.axon_hooks'] = mod


_install_ntff_hook()

import concourse.bass as bass
import concourse.mybir as mybir
import concourse.tile as tile
from concourse import library_config
from concourse.bass_utils import run_bass_kernel_spmd
from concourse.vector_clock import VectorClock, ScopedClock

bf16 = mybir.dt.bfloat16
f32 = mybir.dt.float32
fp8 = mybir.dt.float8e4

# ------------- tile framework patches (walrus: 1 sync wait / inst) ---------


def _drain_and_barrier_split(self, tick_clock, wait_clock):
    nc = self.nc
    full = tick_clock.global_clock
    procs = [p for p in range(27) if full[p] > 0]
    for p in procs:
        sub = VectorClock([full[q] if q == p else 0 for q in range(27)])
        drain_inst = nc.sync.drain(fusable=False)
        wait_clock.add_sem_waits(drain_inst.ins, ScopedClock({None: sub}))
    if not procs:
        nc.sync.drain(fusable=False)
    nc.all_engine_barrier()
    assert self.sems is not None
    popped = nc._tile_sem_poison_stack.pop()
    assert popped is self._sem_poison
    nc.clear_and_free_semaphores(list(self.sems.allocated().values()))
    nc.all_engine_barrier()


def _split_excess_waits(nc):
    for bb in nc.main_func.blocks:
        insts = bb.instructions
        i = 0
        while i < len(insts):
            ins = insts[i]
            si = ins.sync_info
            if si is None:
                i += 1
                continue
            waits = list(si.on_wait)
            if len(waits) <= 1:
                i += 1
                continue
            keep, surplus = waits[:1], waits[1:]
            ins.sync_info = mybir.SyncInfo(on_wait=keep, on_update=list(si.on_update))
            nops = []
            for w in surplus:
                nop = mybir.InstNoOp(name=nc.get_next_instruction_name())
                nop.engine = ins.engine
                nop.sync_info = mybir.SyncInfo(on_wait=[w], on_update=[])
                nc.register_instruction(nop)
                nops.append(nop)
            for k, nop in enumerate(nops):
                insts.insert(i + k, nop)
            i += 1 + len(nops)


_PATCHED = False


def _install_patches():
    global _PATCHED
    if _PATCHED:
        return
    _orig_exit = tile.TileContext.__exit__

    def _exit_with_split(self, *a):
        r = _orig_exit(self, *a)
        _split_excess_waits(self.nc)
        return r

    tile.TileContext._drain_and_barrier = _drain_and_barrier_split
    tile.TileContext.__exit__ = _exit_with_split
    _PATCHED = True


# ---------------- host-side graph preprocessing ----------------


def _wrap16(flat_idx):
    W = len(flat_idx) // 16
    t = np.asarray(flat_idx, np.int16).reshape(W, 16).T
    return np.tile(t, (8, 1))


def _host_prep(x, edge_index, W1, att_src1, att_dst1, b1, W2, att_src2,
               att_dst2, b2):
    src_o = np.concatenate([np.asarray(edge_index[0]),
                            np.arange(N, dtype=np.int64)]).astype(np.int64)
    dst_o = np.concatenate([np.asarray(edge_index[1]),
                            np.arange(N, dtype=np.int64)]).astype(np.int64)

    core_of = dst_o // NPC
    deg = np.bincount(dst_o, minlength=N)

    gid = np.full(N, -1, np.int64)
    node_of_gid = np.full(NGLOB, -1, np.int64)
    for c in range(NCORES):
        nodes = np.arange(c * NPC, (c + 1) * NPC)
        d = deg[nodes]
        order = np.argsort(-d, kind='stable')
        blk_cnt = np.zeros(NB, np.int64)
        blk_load = np.zeros(NB, np.int64)
        assign = np.full(NPC, -1, np.int64)
        for i in order:
            b = int(np.argmin(blk_load + (blk_cnt >= BLK) * (1 << 40)))
            assign[i] = b
            blk_cnt[b] += 1
            blk_load[b] += d[i]
        slot_ctr = np.zeros(NB, np.int64)
        for i in range(NPC):
            b = assign[i]
            g = c * NLOC + b * BLK + slot_ctr[b]
            slot_ctr[b] += 1
            gid[nodes[i]] = g
            node_of_gid[g] = nodes[i]

    src_g = gid[src_o]
    dst_g = gid[dst_o]
    dst_block = (dst_g % NLOC) // BLK
    dst_slot = dst_g % BLK
    src_core = src_g // NLOC
    src_row = src_g % NLOC
    side = (src_row >= R0).astype(np.int64)
    src_sidx = np.where(side == 0, src_core * R0 + src_row,
                        src_core * R1 + (src_row - R0))

    idxL1 = np.full((NCORES, NB, 2, SIDE), -1, np.int32)
    dslL1 = np.full((NCORES, NB, 2, SIDE), -1, np.int32)
    cntL1 = np.zeros((NCORES, NB, 2), np.int64)

    for c in range(NCORES):
        em = np.nonzero(core_of == c)[0]
        eb = dst_block[em]
        for b in range(NB):
            eidx = em[eb == b]
            dslots = dst_slot[eidx]
            sides = side[eidx]
            for s in (0, 1):
                ms = sides == s
                k = int(ms.sum())
                if k > SIDE:
                    raise RuntimeError(f"L1 overflow c{c} b{b} s{s}: {k}")
                idxL1[c, b, s, :k] = src_sidx[eidx][ms]
                dslL1[c, b, s, :k] = dslots[ms]
                cntL1[c, b, s] = k

    # SPMD: same valid-descriptor count on every core -> pad with idx 0
    nvmax = cntL1.max(axis=0)      # [NB, 2]
    nvL1 = np.minimum(((nvmax + 63) // 64) * 64, SIDE)
    for c in range(NCORES):
        for b in range(NB):
            for s in (0, 1):
                k = int(cntL1[c, b, s])
                idxL1[c, b, s, k:int(nvL1[b, s])] = 0

    SL1 = np.zeros((NCORES, NB, 128, CH1 * 128), np.float32)
    STL1 = np.zeros((NCORES, NB, 128, CH1 * 128), np.float32)
    for c in range(NCORES):
        for b in range(NB):
            for s in (0, 1):
                for j in range(CPS):
                    ch = s * CPS + j
                    sl = dslL1[c, b, s, j * 128:(j + 1) * 128]
                    e_i = np.nonzero(sl >= 0)[0]
                    n_i = sl[e_i]
                    SL1[c, b, e_i, ch * 128 + n_i] = 1
                    STL1[c, b, n_i, ch * 128 + e_i] = 1
    SB1 = np.concatenate([SL1, STL1], axis=-1).astype(
        ml_dtypes.float8_e4m3).view(np.int8)  # [NCORES, NB, 128, 2*CH1*128]

    W1 = np.asarray(W1, np.float32)
    att_src1 = np.asarray(att_src1, np.float32)
    att_dst1 = np.asarray(att_dst1, np.float32)
    b1 = np.asarray(b1, np.float32)
    W2 = np.asarray(W2, np.float32)
    att_src2 = np.asarray(att_src2, np.float32)
    att_dst2 = np.asarray(att_dst2, np.float32)
    b2 = np.asarray(b2, np.float32)

    Msrc = np.zeros((F_IN, HEADS), np.float32)
    Mdst = np.zeros((F_IN, HEADS), np.float32)
    for h in range(HEADS):
        Msrc[h * HID:(h + 1) * HID, h] = att_src1[h]
        Mdst[h * HID:(h + 1) * HID, h] = att_dst1[h]
    # h features stored head-major (natural W1 layout: col = head*64 + ch)
    W1aug = np.zeros((F_IN + 128, F_IN + 16), np.float32)
    W1aug[:F_IN, 0:F_IN] = W1
    W1aug[:F_IN, F_IN:F_IN + 8] = W1 @ Msrc
    W1aug[:F_IN, F_IN + 8:F_IN + 16] = W1 @ Mdst
    W1aug[F_IN, 0:F_IN] = b1
    W2aug = np.zeros((F_IN, 69), np.float32)
    W2aug[:, 0:CLS] = W2
    W2aug[:, 64] = W2 @ att_src2[0]
    W2aug[:, 65] = W2 @ att_dst2[0]
    W2bias = np.zeros((128, 69), np.float32)
    W2bias[0, 0:CLS] = b2
    W2bias[0, 68] = 1.0

    x = np.asarray(x, np.float32)
    xTs = []
    for c in range(NCORES):
        cols = node_of_gid[c * NLOC:(c + 1) * NLOC]
        validc = cols >= 0
        xc = np.zeros((NLOC, F_IN), np.float32)
        xc[validc] = x[cols[validc]]
        xt = np.zeros((F_IN + 128, NLOC), np.float32)
        xt[:F_IN] = xc.T
        xt[F_IN] = 1.0
        xTs.append(xt.astype(ml_dtypes.bfloat16))

    idxL1_dev = np.zeros((NCORES, 128, NB * 2 * WCI), np.int16)
    for c in range(NCORES):
        for b in range(NB):
            for s in (0, 1):
                col = (b * 2 + s) * WCI
                idxL1_dev[c, :, col:col + WCI] = _wrap16(idxL1[c, b, s, :])

    in_maps = []
    W1aug_bf = W1aug.astype(ml_dtypes.bfloat16)
    W2aug_bf = W2aug.astype(ml_dtypes.bfloat16)
    W2bias_bf = W2bias.astype(ml_dtypes.bfloat16)
    for c in range(NCORES):
        in_maps.append({
            "xT": np.ascontiguousarray(xTs[c]),
            "W1aug": W1aug_bf,
            "W2aug": W2aug_bf,
            "W2bias": W2bias_bf,
            "idxL1": np.ascontiguousarray(idxL1_dev[c]),
            "SB1": np.ascontiguousarray(SB1[c]),
        })
    meta = {"node_of_gid": node_of_gid, "nvL1": nvL1}
    return in_maps, meta


# ---------------- device program ----------------


def _build_program(nvL1):
    _install_patches()
    nc = bass.Bass(num_swdge_queues=4)
    AF = mybir.ActivationFunctionType
    OP = mybir.AluOpType
    KW = (F_IN + 128) // 128          # 5 k-chunks for GEMM1
    WROW = F_IN + 16                  # 528 W1aug cols

    xT = nc.dram_tensor("xT", [F_IN + 128, NLOC], bf16, kind="ExternalInput")
    W1a = nc.dram_tensor("W1aug", [F_IN + 128, WROW], bf16, kind="ExternalInput")
    W2a = nc.dram_tensor("W2aug", [F_IN, 69], bf16, kind="ExternalInput")
    W2b = nc.dram_tensor("W2bias", [128, 69], bf16, kind="ExternalInput")
    idxL1 = nc.dram_tensor("idxL1", [128, NB * 2 * WCI], mybir.dt.int16,
                           kind="ExternalInput")
    SB1 = nc.dram_tensor("SB1", [NB, 128, 2 * CH1 * 128],
                         mybir.dt.int8, kind="ExternalInput")

    out_cat = nc.dram_tensor("out_cat", [NLOC, 128], f32, kind="ExternalOutput")

    Hcat_loc = nc.dram_tensor("Hcat_loc", [NLOC, HROW], bf16)
    Hcat_g = nc.dram_tensor("Hcat_g", [NGLOB, HROW], bf16, addr_space="Shared")
    Gcat_loc = nc.dram_tensor("Gcat_loc", [NLOC, GROW], bf16)
    Gcat_g = nc.dram_tensor("Gcat_g", [NGLOB, GROW], bf16, addr_space="Shared")

    groups = [list(range(NCORES))]

    with tile.TileContext(nc) as tc:
        with tc.tile_critical():
            nc.gpsimd.load_library(library_config.mlp)
        nvset = sorted({int(v) for v in nvL1.flatten()})
        nvregs = {}
        for v in nvset:
            if v > 0:
                nvregs[v] = nc.gpsimd.to_reg(v)

        with tc.tile_pool(name="const", bufs=1) as constp:
            w1t = constp.tile([128, KW * WROW], bf16)
            for k in range(KW):
                nc.sync.dma_start(w1t[:, k * WROW:(k + 1) * WROW],
                                  W1a[k * 128:(k + 1) * 128, :])
            w2t = constp.tile([128, 4 * 69], bf16)
            for k in range(4):
                nc.sync.dma_start(w2t[:, k * 69:(k + 1) * 69],
                                  W2a[k * 128:(k + 1) * 128, :])
            w2bt = constp.tile([128, 69], bf16)
            nc.sync.dma_start(w2bt[:], W2b[:, :])
            e0ones = constp.tile([128, 128], bf16)
            nc.vector.memset(e0ones[:], 0.0)
            nc.vector.memset(e0ones[0:1, :], 1.0)
            ident = constp.tile([128, 128], bf16)
            from concourse.masks import make_identity
            make_identity(nc, ident[:])
            dt_all = constp.tile([128, NB * 16], bf16)
            dt2_all = constp.tile([128, NB * 2], bf16)
            idx1t = constp.tile([128, NB * 2 * WCI], mybir.dt.int16)
            nc.sync.dma_start(idx1t[:], idxL1[:, :])

            # ---- phase 1: GEMM1 + Hcat rows ----
            with tc.tile_pool(name="p1", bufs=3) as p1, \
                 tc.tile_pool(name="ps1a", bufs=2, space="PSUM") as ps1a, \
                 tc.tile_pool(name="ps1b", bufs=2, space="PSUM") as ps1b:
                for b in range(NB):
                    pA = ps1a.tile([128, F_IN], f32)
                    pB = ps1b.tile([128, 16], f32)
                    xt = p1.tile([128, KW * 128], bf16, tag="xt")
                    nc.sync.dma_start(
                        xt[:].rearrange("p (k j) -> p k j", k=KW),
                        xT[0:KW * 128, b * 128:(b + 1) * 128].rearrange(
                            "(k p) j -> p k j", k=KW))
                    for k in range(KW):
                        nc.tensor.matmul(pA[:], xt[:, k * 128:(k + 1) * 128],
                                         w1t[:, k * WROW:k * WROW + F_IN],
                                         start=(k == 0), stop=(k == KW - 1))
                        nc.tensor.matmul(pB[:], xt[:, k * 128:(k + 1) * 128],
                                         w1t[:, k * WROW + F_IN:(k + 1) * WROW],
                                         start=(k == 0), stop=(k == KW - 1))
                    hc = p1.tile([128, HROW], bf16, tag="hc")
                    nc.vector.tensor_copy(hc[:, 0:F_IN], pA[:])
                    nc.scalar.activation(hc[:, 512:520], pB[:, 0:8], AF.Exp)
                    nc.scalar.activation(hc[:, 520:528], pB[:, 0:8], AF.Exp,
                                         scale=NEG_SLOPE)
                    nc.scalar.activation(hc[:, 528:536], pB[:, 8:16], AF.Exp)
                    nc.scalar.activation(hc[:, 536:544], pB[:, 8:16], AF.Exp,
                                         scale=NEG_SLOPE)
                    if b < 3:
                        nc.vector.memset(hc[:, 544:HROW], 0.0)
                    nc.vector.tensor_copy(dt_all[:, b * 16:(b + 1) * 16],
                                          hc[:, 528:544])
                    nc.scalar.dma_start(Hcat_loc[b * 128:(b + 1) * 128, :], hc[:])
                    if b == AGS0 - 1:
                        nc.gpsimd.collective_compute(
                            "AllGather", mybir.AluOpType.bypass,
                            replica_groups=groups,
                            ins=[Hcat_loc[0:R0, :]],
                            outs=[Hcat_g[0:S0TOT, :]])

            # ---- phase 2: AllGather Hcat (tail chunk) ----
            nc.gpsimd.collective_compute(
                "AllGather", mybir.AluOpType.bypass, replica_groups=groups,
                ins=[Hcat_loc[R0:NLOC, :]],
                outs=[Hcat_g[S0TOT:NGLOB, :]])

            # ---- phase 3: L1 edges + block tails + GEMM2 + Gcat ----
            with tc.tile_pool(name="p3", bufs=3) as p3, \
                 tc.tile_pool(name="pg3", bufs=1) as pg3, \
                 tc.tile_pool(name="p3s", bufs=2) as p3s, \
                 tc.tile_pool(name="psA", bufs=2, space="PSUM") as psA, \
                 tc.tile_pool(name="psB", bufs=2, space="PSUM") as psB, \
                 tc.tile_pool(name="psuv", bufs=1, space="PSUM") as psuv, \
                 tc.tile_pool(name="pst", bufs=1, space="PSUM") as pst, \
                 tc.tile_pool(name="ps2", bufs=1, space="PSUM") as ps2:
                for b in range(NB):
                    sbst = p3s.tile([128, 2 * CH1 * 128], mybir.dt.int8,
                                    tag="sbst")
                    nc.sync.dma_start(sbst[:], SB1[b, :, :])
                    sb = sbst[:, 0:CH1 * 128].bitcast(fp8)
                    stb = sbst[:, CH1 * 128:2 * CH1 * 128].bitcast(fp8)
                    gl = []
                    for s in (0, 1):
                        g = pg3.tile([128, CPS * HROW], bf16, tag=f"g{s}{b % 3}")
                        gv = g[:].rearrange("p (c w) -> p c w", w=HROW)
                        if b < 3:
                            nc.vector.memset(g[:], 0.0)
                        nvs = int(nvL1[b, s])
                        if nvs > 0:
                            nc.gpsimd.dma_gather(
                                out_ap=gv[:, :, :],
                                in_ap=(Hcat_g[0:S0TOT, :] if s == 0 else
                                       Hcat_g[S0TOT:NGLOB, :]),
                                idxs_ap=idx1t[:, (b * 2 + s) * WCI:
                                              (b * 2 + s + 1) * WCI],
                                num_idxs=SIDE, num_idxs_reg=nvregs[nvs],
                                elem_size=HROW, queue_num=(b * 2 + s) % 4)
                        gl.append(g)
                    puv = psuv.tile([128, CH1 * 16], f32)
                    for c in range(CH1):
                        nc.tensor.matmul(puv[:, c * 16:(c + 1) * 16],
                                         stb[:, c * 128:(c + 1) * 128],
                                         dt_all[:, b * 16:(b + 1) * 16],
                                         start=True, stop=True)
                    uvb = p3.tile([128, CH1 * 16], bf16, tag="uvb")
                    nc.scalar.activation(uvb[:], puv[:], AF.Copy)
                    pA = psA.tile([128, F_IN], f32)
                    pB = psB.tile([128, 8], f32)
                    hs = p3.tile([128, CH1 * 520], bf16, tag="hs")
                    hs3 = hs[:].rearrange("p (c w) -> p c w", w=520)
                    uvs = uvb[:].rearrange("p (c w) -> p c w", w=16)
                    for s in (0, 1):
                        g3 = gl[s][:].rearrange("p (c w) -> p c w", w=HROW)
                        tU = p3.tile([128, CPS * 8], f32, tag="tU")
                        tV = p3.tile([128, CPS * 8], f32, tag="tV")
                        exb = p3.tile([128, CPS * 8], bf16, tag=f"exb{s}")
                        nc.vector.tensor_tensor(
                            out=tU[:].rearrange("p (c w) -> p c w", w=8),
                            in0=g3[:, :, 512:520],
                            in1=uvs[:, s * CPS:(s + 1) * CPS, 0:8],
                            op=OP.mult)
                        nc.vector.tensor_tensor(
                            out=tV[:].rearrange("p (c w) -> p c w", w=8),
                            in0=g3[:, :, 520:528],
                            in1=uvs[:, s * CPS:(s + 1) * CPS, 8:16],
                            op=OP.mult)
                        nc.vector.tensor_tensor(out=exb[:], in0=tU[:], in1=tV[:],
                                                op=OP.max)
                        nc.vector.tensor_tensor(
                            out=hs3[:, s * CPS:(s + 1) * CPS, 0:F_IN].rearrange(
                                "p c (h w) -> p c h w", h=HEADS),
                            in0=g3[:, :, 0:F_IN].rearrange(
                                "p c (h w) -> p c h w", h=HEADS),
                            in1=exb[:].rearrange("p (c h o) -> p c h o",
                                                 h=HEADS, o=1).to_broadcast(
                                [128, CPS, HEADS, HID]),
                            op=OP.mult)
                        nc.vector.tensor_copy(
                            hs3[:, s * CPS:(s + 1) * CPS, 512:520],
                            exb[:].rearrange("p (c w) -> p c w", w=8))
                    for c in range(CH1):
                        nc.tensor.matmul(pA[:], sb[:, c * 128:(c + 1) * 128],
                                         hs3[:, c, 0:F_IN],
                                         start=(c == 0), stop=(c == CH1 - 1))
                        nc.tensor.matmul(pB[:], sb[:, c * 128:(c + 1) * 128],
                                         hs3[:, c, 512:520],
                                         start=(c == 0), stop=(c == CH1 - 1))
                    sden = p3.tile([128, 8], f32, tag="sden")
                    nc.vector.tensor_scalar_add(sden[:], pB[:], EPS)
                    rec = p3.tile([128, 8], f32, tag="rec")
                    nc.vector.reciprocal(rec[:], sden[:])
                    o1 = p3.tile([128, F_IN], f32, tag="o1")
                    nc.vector.tensor_tensor(
                        out=o1[:].rearrange("p (h w) -> p h w", h=HEADS),
                        in0=pA[:].rearrange("p (h w) -> p h w", h=HEADS),
                        in1=rec[:].rearrange("p (h o) -> p h o", o=1).to_broadcast(
                            [128, HEADS, HID]),
                        op=OP.mult)
                    rneg = p3.tile([128, F_IN], f32, tag="rneg")
                    nc.scalar.activation(rneg[:], o1[:], AF.Relu, scale=-1.0)
                    eneg = p3.tile([128, F_IN], f32, tag="eneg")
                    nc.scalar.activation(eneg[:], rneg[:], AF.Exp, scale=-1.0)
                    x1 = p3.tile([128, F_IN], f32, tag="x1")
                    nc.vector.tensor_scalar_add(x1[:], o1[:], 1.0)
                    m1 = p3.tile([128, F_IN], f32, tag="m1")
                    nc.vector.tensor_tensor(out=m1[:], in0=x1[:], in1=eneg[:],
                                            op=OP.max)
                    h1b = p3.tile([128, F_IN], bf16, tag="h1b")
                    nc.vector.tensor_scalar_add(h1b[:], m1[:], -1.0)
                    p2t = ps2.tile([128, 69], f32)
                    for k in range(4):
                        ptt = pst.tile([128, 128], bf16)
                        nc.tensor.transpose(ptt[:], h1b[:, k * 128:(k + 1) * 128],
                                            ident[:])
                        h1tk = p3.tile([128, 128], bf16, tag="h1tk")
                        nc.scalar.activation(h1tk[:], ptt[:], AF.Copy)
                        nc.tensor.matmul(p2t[:], h1tk[:],
                                         w2t[:, k * 69:(k + 1) * 69],
                                         start=(k == 0), stop=False)
                    nc.tensor.matmul(p2t[:], e0ones[:], w2bt[:],
                                     start=False, stop=True)
                    gt = p3.tile([128, GROW], bf16, tag="gt")
                    nc.scalar.activation(gt[:, 0:64], p2t[:, 0:64], AF.Copy)
                    nc.scalar.activation(gt[:, 64:65], p2t[:, 64:65], AF.Exp)
                    nc.scalar.activation(gt[:, 65:66], p2t[:, 64:65], AF.Exp,
                                         scale=NEG_SLOPE)
                    nc.scalar.activation(gt[:, 66:67], p2t[:, 65:66], AF.Exp)
                    nc.scalar.activation(gt[:, 67:68], p2t[:, 65:66], AF.Exp,
                                         scale=NEG_SLOPE)
                    nc.scalar.activation(gt[:, 68:69], p2t[:, 68:69], AF.Copy)
                    if b < 3:
                        nc.vector.memset(gt[:, 69:GROW], 0.0)
                    nc.vector.tensor_copy(dt2_all[:, b * 2:(b + 1) * 2],
                                          gt[:, 66:68])
                    nc.scalar.dma_start(Gcat_loc[b * 128:(b + 1) * 128, :], gt[:])
                    if b == AGS0 - 1:
                        nc.gpsimd.collective_compute(
                            "AllGather", mybir.AluOpType.bypass,
                            replica_groups=groups,
                            ins=[Gcat_loc[0:R0, :]],
                            outs=[Gcat_g[0:S0TOT, :]])

            # ---- phase 4: AllGather Gcat (tail chunk) ----
            nc.gpsimd.collective_compute(
                "AllGather", mybir.AluOpType.bypass, replica_groups=groups,
                ins=[Gcat_loc[R0:NLOC, :]],
                outs=[Gcat_g[S0TOT:NGLOB, :]])

            # ---- phase 5: L2 edges + log_softmax ----
            with tc.tile_pool(name="p5", bufs=3) as p5, \
                 tc.tile_pool(name="pg5", bufs=1) as pg5, \
                 tc.tile_pool(name="p5s", bufs=2) as p5s, \
                 tc.tile_pool(name="psuv2", bufs=1, space="PSUM") as psuv2, \
                 tc.tile_pool(name="ps2b", bufs=2, space="PSUM") as ps2b:
                for b in range(NB):
                    sbst5 = p5s.tile([128, 2 * CH1 * 128], mybir.dt.int8,
                                     tag="sbst5")
                    nc.sync.dma_start(sbst5[:], SB1[b, :, :])
                    sb5 = sbst5[:, 0:CH1 * 128].bitcast(fp8)
                    st5 = sbst5[:, CH1 * 128:2 * CH1 * 128].bitcast(fp8)
                    g4 = pg5.tile([128, CH1 * GROW], bf16, tag=f"g4{b % 3}")
                    g4r = g4[:].rearrange("p (c w) -> p c w", w=GROW)
                    if b < 3:
                        nc.vector.memset(g4[:], 0.0)
                    for s in (0, 1):
                        nvs = int(nvL1[b, s])
                        if nvs > 0:
                            nc.gpsimd.dma_gather(
                                out_ap=g4r[:, s * CPS:(s + 1) * CPS, :],
                                in_ap=(Gcat_g[0:S0TOT, :] if s == 0 else
                                       Gcat_g[S0TOT:NGLOB, :]),
                                idxs_ap=idx1t[:, (b * 2 + s) * WCI:
                                              (b * 2 + s + 1) * WCI],
                                num_idxs=SIDE, num_idxs_reg=nvregs[nvs],
                                elem_size=GROW, queue_num=(b * 2 + s) % 4)
                    puv2 = psuv2.tile([128, CH1 * 2], f32)
                    for c in range(CH1):
                        nc.tensor.matmul(puv2[:, c * 2:(c + 1) * 2],
                                         st5[:, c * 128:(c + 1) * 128],
                                         dt2_all[:, b * 2:(b + 1) * 2],
                                         start=True, stop=True)
                    uv2b = p5.tile([128, CH1 * 2], bf16, tag="uv2b")
                    nc.scalar.activation(uv2b[:], puv2[:], AF.Copy)
                    uv2v = uv2b[:].rearrange("p (c w) -> p c w", w=2)
                    U2 = p5.tile([128, CH1], f32, tag="U2")
                    V2 = p5.tile([128, CH1], f32, tag="V2")
                    nc.vector.tensor_tensor(
                        out=U2[:].rearrange("p (c o) -> p c o", o=1),
                        in0=g4r[:, :, 64:65], in1=uv2v[:, :, 0:1], op=OP.mult)
                    nc.vector.tensor_tensor(
                        out=V2[:].rearrange("p (c o) -> p c o", o=1),
                        in0=g4r[:, :, 65:66], in1=uv2v[:, :, 1:2], op=OP.mult)
                    ex2 = p5.tile([128, CH1], bf16, tag="ex2")
                    nc.vector.tensor_tensor(out=ex2[:], in0=U2[:], in1=V2[:],
                                            op=OP.max)
                    g4f = p5.tile([128, CH1 * GROW], bf16, tag="g4f")
                    nc.vector.tensor_tensor(
                        out=g4f[:].rearrange("p (c w) -> p c w", w=GROW),
                        in0=g4r,
                        in1=ex2[:].rearrange("p (c o) -> p c o", o=1).to_broadcast(
                            [128, CH1, GROW]),
                        op=OP.mult)
                    g4f3 = g4f[:].rearrange("p (c w) -> p c w", w=GROW)
                    p2b = ps2b.tile([128, 69], f32)
                    for c in range(CH1):
                        nc.tensor.matmul(p2b[:], sb5[:, c * 128:(c + 1) * 128],
                                         g4f3[:, c, 0:69],
                                         start=(c == 0), stop=(c == CH1 - 1))
                    den2 = p5.tile([128, 1], f32, tag="den2")
                    nc.vector.tensor_scalar_add(den2[:], p2b[:, 68:69], EPS)
                    rec2 = p5.tile([128, 1], f32, tag="rec2")
                    nc.vector.reciprocal(rec2[:], den2[:])
                    ozt = p5.tile([128, 128], f32, tag="ozt")
                    Zt = ozt[:, 64:128]
                    nc.vector.tensor_scalar_mul(Zt, p2b[:, 0:64], rec2[:, 0:1])
                    mx = p5.tile([128, 1], f32, tag="mx")
                    nc.vector.reduce_max(mx[:], Zt, axis=mybir.AxisListType.X)
                    nmx = p5.tile([128, 1], f32, tag="nmx")
                    nc.vector.tensor_scalar_mul(nmx[:], mx[:], -1.0)
                    ez = p5.tile([128, 64], f32, tag="ez")
                    nc.scalar.activation(ez[:], Zt, AF.Exp, bias=nmx[:, 0:1])
                    sz = p5.tile([128, 1], f32, tag="sz")
                    nc.vector.reduce_sum(sz[:], ez[:], axis=mybir.AxisListType.X)
                    lse = p5.tile([128, 1], f32, tag="lse")
                    nc.scalar.activation(lse[:], sz[:], AF.Ln)
                    nc.vector.tensor_scalar(
                        out=ozt[:, 0:64], in0=Zt, scalar1=mx[:, 0:1],
                        scalar2=lse[:, 0:1], op0=OP.subtract, op1=OP.subtract)
                    nc.scalar.dma_start(out_cat[b * 128:(b + 1) * 128, :], ozt[:])

    mybir.codegen_inst_isa_subclasses(nc)
    return nc


# ---------------- top-level entry ----------------

_CACHE = {}


def kernel(x, edge_index, W1, att_src1, att_dst1, b1, W2, att_src2, att_dst2,
           b2, _trace=False):
    in_maps, meta = _host_prep(x, edge_index, W1, att_src1, att_dst1, b1, W2,
                               att_src2, att_dst2, b2)
    if "prog" not in _CACHE:
        _CACHE["prog"] = _build_program(meta["nvL1"])
    nc = _CACHE["prog"]
    res = run_bass_kernel_spmd(nc, in_maps, list(range(NCORES)), trace=_trace)
    node_of_gid = meta["node_of_gid"]
    out = np.zeros((N, 64), np.float32)
    Z = np.zeros((N, 64), np.float32)
    for c in range(NCORES):
        oc = np.asarray(res.results[c]["out_cat"])
        cols = node_of_gid[c * NLOC:(c + 1) * NLOC]
        valid = cols >= 0
        out[cols[valid]] = oc[valid, 0:64]
        Z[cols[valid]] = oc[valid, 64:128]
    kernel._last_exec_ns = res.exec_time_ns
    kernel._last_res = res
    return (out, Z)


# revision 12
# speedup vs baseline: 3.8961x; 1.0515x over previous
"""GAT 2-layer kernel for 8 trn2 NeuronCores (self-contained).

Sharding: destination-node blocks across 8 cores. Per core: 49 blocks x 128
nodes. Layer GEMMs computed on owned nodes; per-node attention factors
(u=exp(a_src), v=exp(0.2 a_src), and dst-side) folded into gatherable row
tables (Hcat/Gcat) that are AllGathered; edge phases gather rows by src via
dma_gather and do segment-softmax-aggregation with per-chunk selection-matrix
matmuls (exp(leaky(x)) == max(exp(x), exp(0.2 x)) makes the logits separable).
Edge slots are split by source half (int16 gather indices); both layers share
the same slot assignment so one selection-matrix pair and one index table
serve both edge phases.
"""
import sys, types
sys.path.insert(0, '/opt/trn_rl_repo')

import numpy as np
import ml_dtypes

# ---------------- problem constants (hardcoded) ----------------
N = 50000
F_IN = 512
HID = 64
HEADS = 8
CLS = 64
NEG_SLOPE = 0.2
NCORES = 8
NPC = 6250
NB = 49
BLK = 128
NLOC = NB * BLK           # 6272
NGLOB = NCORES * NLOC     # 50176
CPS = 6                   # chunks per side per block
SIDE = CPS * 128          # 768 edge slots per side
CH1 = 2 * CPS             # 12 chunks per block
WCI = SIDE // 16          # 48 idx cols per side
HROW = 640                # Hcat row (bf16)
GROW = 128                # Gcat row (bf16)
AGS0 = 25                 # blocks per core in AG chunk A
R0 = AGS0 * BLK           # 3200 rows per core in chunk A
R1 = NLOC - R0            # 3072 rows per core in chunk B
S0TOT = NCORES * R0       # 25600 rows in side/chunk 0
S1TOT = NCORES * R1       # 24576 rows in side/chunk 1
EPS = 1e-16


def _install_ntff_hook():
    if 'antenv.axon_hooks' in sys.modules:
        return
    try:
        sys.path.insert(0, '/root/.axon_site')
        from trn_agent_boot.trn_boot import _ntff_profile_via_ctypes
        hook = _ntff_profile_via_ctypes('/opt/axon/libaxon_pjrt.so')
    except Exception:
        return
    mod = types.ModuleType('antenv.axon_hooks')
    mod._hook = hook
    mod.get_axon_ntff_profile_hook = lambda: mod._hook
    mod.set_axon_ntff_profile_hook = lambda h: setattr(mod, '_hook', h)
    sys.modules['antenv.axon_hooks'] = mod


_install_ntff_hook()

import concourse.bass as bass
import concourse.mybir as mybir
import concourse.tile as tile
from concourse import library_config
from concourse.bass_utils import run_bass_kernel_spmd
from concourse.vector_clock import VectorClock, ScopedClock

bf16 = mybir.dt.bfloat16
f32 = mybir.dt.float32
fp8 = mybir.dt.float8e4

# ------------- tile framework patches (walrus: 1 sync wait / inst) ---------


def _drain_and_barrier_split(self, tick_clock, wait_clock):
    nc = self.nc
    full = tick_clock.global_clock
    procs = [p for p in range(27) if full[p] > 0]
    for p in procs:
        sub = VectorClock([full[q] if q == p else 0 for q in range(27)])
        drain_inst = nc.sync.drain(fusable=False)
        wait_clock.add_sem_waits(drain_inst.ins, ScopedClock({None: sub}))
    if not procs:
        nc.sync.drain(fusable=False)
    nc.all_engine_barrier()
    assert self.sems is not None
    popped = nc._tile_sem_poison_stack.pop()
    assert popped is self._sem_poison
    nc.clear_and_free_semaphores(list(self.sems.allocated().values()))
    nc.all_engine_barrier()


def _split_excess_waits(nc):
    for bb in nc.main_func.blocks:
        insts = bb.instructions
        i = 0
        while i < len(insts):
            ins = insts[i]
            si = ins.sync_info
            if si is None:
                i += 1
                continue
            waits = list(si.on_wait)
            if len(waits) <= 1:
                i += 1
                continue
            keep, surplus = waits[:1], waits[1:]
            ins.sync_info = mybir.SyncInfo(on_wait=keep, on_update=list(si.on_update))
            nops = []
            for w in surplus:
                nop = mybir.InstNoOp(name=nc.get_next_instruction_name())
                nop.engine = ins.engine
                nop.sync_info = mybir.SyncInfo(on_wait=[w], on_update=[])
                nc.register_instruction(nop)
                nops.append(nop)
            for k, nop in enumerate(nops):
                insts.insert(i + k, nop)
            i += 1 + len(nops)


_PATCHED = False


def _install_patches():
    global _PATCHED
    if _PATCHED:
        return
    _orig_exit = tile.TileContext.__exit__

    def _exit_with_split(self, *a):
        r = _orig_exit(self, *a)
        _split_excess_waits(self.nc)
        return r

    tile.TileContext._drain_and_barrier = _drain_and_barrier_split
    tile.TileContext.__exit__ = _exit_with_split
    _PATCHED = True


# ---------------- host-side graph preprocessing ----------------


def _wrap16(flat_idx):
    W = len(flat_idx) // 16
    t = np.asarray(flat_idx, np.int16).reshape(W, 16).T
    return np.tile(t, (8, 1))


def _host_prep(x, edge_index, W1, att_src1, att_dst1, b1, W2, att_src2,
               att_dst2, b2):
    src_o = np.concatenate([np.asarray(edge_index[0]),
                            np.arange(N, dtype=np.int64)]).astype(np.int64)
    dst_o = np.concatenate([np.asarray(edge_index[1]),
                            np.arange(N, dtype=np.int64)]).astype(np.int64)

    core_of = dst_o // NPC
    deg = np.bincount(dst_o, minlength=N)

    gid = np.full(N, -1, np.int64)
    node_of_gid = np.full(NGLOB, -1, np.int64)
    for c in range(NCORES):
        nodes = np.arange(c * NPC, (c + 1) * NPC)
        d = deg[nodes]
        order = np.argsort(-d, kind='stable')
        blk_cnt = np.zeros(NB, np.int64)
        blk_load = np.zeros(NB, np.int64)
        assign = np.full(NPC, -1, np.int64)
        for i in order:
            b = int(np.argmin(blk_load + (blk_cnt >= BLK) * (1 << 40)))
            assign[i] = b
            blk_cnt[b] += 1
            blk_load[b] += d[i]
        slot_ctr = np.zeros(NB, np.int64)
        for i in range(NPC):
            b = assign[i]
            g = c * NLOC + b * BLK + slot_ctr[b]
            slot_ctr[b] += 1
            gid[nodes[i]] = g
            node_of_gid[g] = nodes[i]

    src_g = gid[src_o]
    dst_g = gid[dst_o]
    dst_block = (dst_g % NLOC) // BLK
    dst_slot = dst_g % BLK
    src_core = src_g // NLOC
    src_row = src_g % NLOC
    side = (src_row >= R0).astype(np.int64)
    src_sidx = np.where(side == 0, src_core * R0 + src_row,
                        src_core * R1 + (src_row - R0))

    idxL1 = np.full((NCORES, NB, 2, SIDE), -1, np.int32)
    dslL1 = np.full((NCORES, NB, 2, SIDE), -1, np.int32)
    cntL1 = np.zeros((NCORES, NB, 2), np.int64)

    for c in range(NCORES):
        em = np.nonzero(core_of == c)[0]
        eb = dst_block[em]
        for b in range(NB):
            eidx = em[eb == b]
            dslots = dst_slot[eidx]
            sides = side[eidx]
            for s in (0, 1):
                ms = sides == s
                k = int(ms.sum())
                if k > SIDE:
                    raise RuntimeError(f"L1 overflow c{c} b{b} s{s}: {k}")
                idxL1[c, b, s, :k] = src_sidx[eidx][ms]
                dslL1[c, b, s, :k] = dslots[ms]
                cntL1[c, b, s] = k

    # SPMD: same valid-descriptor count on every core -> pad with idx 0
    nvmax = cntL1.max(axis=0)      # [NB, 2]
    nvL1 = np.minimum(((nvmax + 63) // 64) * 64, SIDE)
    for c in range(NCORES):
        for b in range(NB):
            for s in (0, 1):
                k = int(cntL1[c, b, s])
                idxL1[c, b, s, k:int(nvL1[b, s])] = 0

    SL1 = np.zeros((NCORES, NB, 128, CH1 * 128), np.float32)
    STL1 = np.zeros((NCORES, NB, 128, CH1 * 128), np.float32)
    for c in range(NCORES):
        for b in range(NB):
            for s in (0, 1):
                for j in range(CPS):
                    ch = s * CPS + j
                    sl = dslL1[c, b, s, j * 128:(j + 1) * 128]
                    e_i = np.nonzero(sl >= 0)[0]
                    n_i = sl[e_i]
                    SL1[c, b, e_i, ch * 128 + n_i] = 1
                    STL1[c, b, n_i, ch * 128 + e_i] = 1
    SB1 = np.concatenate([SL1, STL1], axis=-1).astype(
        ml_dtypes.float8_e4m3).view(np.int8)  # [NCORES, NB, 128, 2*CH1*128]

    W1 = np.asarray(W1, np.float32)
    att_src1 = np.asarray(att_src1, np.float32)
    att_dst1 = np.asarray(att_dst1, np.float32)
    b1 = np.asarray(b1, np.float32)
    W2 = np.asarray(W2, np.float32)
    att_src2 = np.asarray(att_src2, np.float32)
    att_dst2 = np.asarray(att_dst2, np.float32)
    b2 = np.asarray(b2, np.float32)

    Msrc = np.zeros((F_IN, HEADS), np.float32)
    Mdst = np.zeros((F_IN, HEADS), np.float32)
    for h in range(HEADS):
        Msrc[h * HID:(h + 1) * HID, h] = att_src1[h]
        Mdst[h * HID:(h + 1) * HID, h] = att_dst1[h]
    # h features stored head-major (natural W1 layout: col = head*64 + ch)
    W1aug = np.zeros((F_IN + 128, F_IN + 16), np.float32)
    W1aug[:F_IN, 0:F_IN] = W1
    W1aug[:F_IN, F_IN:F_IN + 8] = W1 @ Msrc
    W1aug[:F_IN, F_IN + 8:F_IN + 16] = W1 @ Mdst
    W1aug[F_IN, 0:F_IN] = b1
    W2aug = np.zeros((F_IN, 69), np.float32)
    W2aug[:, 0:CLS] = W2
    W2aug[:, 64] = W2 @ att_src2[0]
    W2aug[:, 65] = W2 @ att_dst2[0]
    W2bias = np.zeros((128, 69), np.float32)
    W2bias[0, 0:CLS] = b2
    W2bias[0, 68] = 1.0

    x = np.asarray(x, np.float32)
    xTs = []
    for c in range(NCORES):
        cols = node_of_gid[c * NLOC:(c + 1) * NLOC]
        validc = cols >= 0
        xc = np.zeros((NLOC, F_IN), np.float32)
        xc[validc] = x[cols[validc]]
        xt = np.zeros((F_IN + 128, NLOC), np.float32)
        xt[:F_IN] = xc.T
        xt[F_IN] = 1.0
        xTs.append(xt.astype(ml_dtypes.bfloat16))

    idxL1_dev = np.zeros((NCORES, 128, NB * 2 * WCI), np.int16)
    for c in range(NCORES):
        for b in range(NB):
            for s in (0, 1):
                col = (b * 2 + s) * WCI
                idxL1_dev[c, :, col:col + WCI] = _wrap16(idxL1[c, b, s, :])

    in_maps = []
    W1aug_bf = W1aug.astype(ml_dtypes.bfloat16)
    W2aug_bf = W2aug.astype(ml_dtypes.bfloat16)
    W2bias_bf = W2bias.astype(ml_dtypes.bfloat16)
    for c in range(NCORES):
        in_maps.append({
            "xT": np.ascontiguousarray(xTs[c]),
            "W1aug": W1aug_bf,
            "W2aug": W2aug_bf,
            "W2bias": W2bias_bf,
            "idxL1": np.ascontiguousarray(idxL1_dev[c]),
            "SB1": np.ascontiguousarray(SB1[c]),
        })
    meta = {"node_of_gid": node_of_gid, "nvL1": nvL1}
    return in_maps, meta


# ---------------- device program ----------------


def _build_program(nvL1):
    _install_patches()
    nc = bass.Bass(num_swdge_queues=4)
    AF = mybir.ActivationFunctionType
    OP = mybir.AluOpType
    KW = (F_IN + 128) // 128          # 5 k-chunks for GEMM1
    WROW = F_IN + 16                  # 528 W1aug cols

    xT = nc.dram_tensor("xT", [F_IN + 128, NLOC], bf16, kind="ExternalInput")
    W1a = nc.dram_tensor("W1aug", [F_IN + 128, WROW], bf16, kind="ExternalInput")
    W2a = nc.dram_tensor("W2aug", [F_IN, 69], bf16, kind="ExternalInput")
    W2b = nc.dram_tensor("W2bias", [128, 69], bf16, kind="ExternalInput")
    idxL1 = nc.dram_tensor("idxL1", [128, NB * 2 * WCI], mybir.dt.int16,
                           kind="ExternalInput")
    SB1 = nc.dram_tensor("SB1", [NB, 128, 2 * CH1 * 128],
                         mybir.dt.int8, kind="ExternalInput")

    out_cat = nc.dram_tensor("out_cat", [NLOC, 128], f32, kind="ExternalOutput")

    Hcat_loc = nc.dram_tensor("Hcat_loc", [NLOC, HROW], bf16)
    Hcat_g = nc.dram_tensor("Hcat_g", [NGLOB, HROW], bf16, addr_space="Shared")
    Gcat_loc = nc.dram_tensor("Gcat_loc", [NLOC, GROW], bf16)
    Gcat_g = nc.dram_tensor("Gcat_g", [NGLOB, GROW], bf16, addr_space="Shared")

    groups = [list(range(NCORES))]

    with tile.TileContext(nc) as tc:
        with tc.tile_critical():
            nc.gpsimd.load_library(library_config.mlp)
        nvset = sorted({int(v) for v in nvL1.flatten()})
        nvregs = {}
        for v in nvset:
            if v > 0:
                nvregs[v] = nc.gpsimd.to_reg(v)

        with tc.tile_pool(name="const", bufs=1) as constp:
            w1t = constp.tile([128, KW * WROW], bf16)
            for k in range(KW):
                nc.sync.dma_start(w1t[:, k * WROW:(k + 1) * WROW],
                                  W1a[k * 128:(k + 1) * 128, :])
            w2t = constp.tile([128, 4 * 69], bf16)
            for k in range(4):
                nc.sync.dma_start(w2t[:, k * 69:(k + 1) * 69],
                                  W2a[k * 128:(k + 1) * 128, :])
            w2bt = constp.tile([128, 69], bf16)
            nc.sync.dma_start(w2bt[:], W2b[:, :])
            e0ones = constp.tile([128, 128], bf16)
            nc.vector.memset(e0ones[:], 0.0)
            nc.vector.memset(e0ones[0:1, :], 1.0)
            ident = constp.tile([128, 128], bf16)
            from concourse.masks import make_identity
            make_identity(nc, ident[:])
            dt_all = constp.tile([128, NB * 16], bf16)
            dt2_all = constp.tile([128, NB * 2], bf16)
            idx1t = constp.tile([128, NB * 2 * WCI], mybir.dt.int16)
            nc.sync.dma_start(idx1t[:], idxL1[:, :])

            # ---- phase 1: GEMM1 + Hcat rows ----
            with tc.tile_pool(name="p1", bufs=3) as p1, \
                 tc.tile_pool(name="ps1a", bufs=2, space="PSUM") as ps1a, \
                 tc.tile_pool(name="ps1b", bufs=2, space="PSUM") as ps1b:
                for b in range(NB):
                    pA = ps1a.tile([128, F_IN], f32)
                    pB = ps1b.tile([128, 16], f32)
                    xt = p1.tile([128, KW * 128], bf16, tag="xt")
                    nc.sync.dma_start(
                        xt[:].rearrange("p (k j) -> p k j", k=KW),
                        xT[0:KW * 128, b * 128:(b + 1) * 128].rearrange(
                            "(k p) j -> p k j", k=KW))
                    for k in range(KW):
                        nc.tensor.matmul(pA[:], xt[:, k * 128:(k + 1) * 128],
                                         w1t[:, k * WROW:k * WROW + F_IN],
                                         start=(k == 0), stop=(k == KW - 1))
                        nc.tensor.matmul(pB[:], xt[:, k * 128:(k + 1) * 128],
                                         w1t[:, k * WROW + F_IN:(k + 1) * WROW],
                                         start=(k == 0), stop=(k == KW - 1))
                    hc = p1.tile([128, HROW], bf16, tag="hc")
                    nc.scalar.activation(hc[:, 0:F_IN], pA[:], AF.Copy)
                    nc.scalar.activation(hc[:, 512:520], pB[:, 0:8], AF.Exp)
                    nc.scalar.activation(hc[:, 520:528], pB[:, 0:8], AF.Exp,
                                         scale=NEG_SLOPE)
                    nc.scalar.activation(hc[:, 528:536], pB[:, 8:16], AF.Exp)
                    nc.scalar.activation(hc[:, 536:544], pB[:, 8:16], AF.Exp,
                                         scale=NEG_SLOPE)
                    if b < 3:
                        nc.vector.memset(hc[:, 544:HROW], 0.0)
                    nc.vector.tensor_copy(dt_all[:, b * 16:(b + 1) * 16],
                                          hc[:, 528:544])
                    nc.scalar.dma_start(Hcat_loc[b * 128:(b + 1) * 128, :], hc[:])
                    if b == AGS0 - 1:
                        nc.gpsimd.collective_compute(
                            "AllGather", mybir.AluOpType.bypass,
                            replica_groups=groups,
                            ins=[Hcat_loc[0:R0, :]],
                            outs=[Hcat_g[0:S0TOT, :]])

            # ---- phase 2: AllGather Hcat (tail chunk) ----
            nc.gpsimd.collective_compute(
                "AllGather", mybir.AluOpType.bypass, replica_groups=groups,
                ins=[Hcat_loc[R0:NLOC, :]],
                outs=[Hcat_g[S0TOT:NGLOB, :]])

            # ---- phase 3: L1 edges + block tails + GEMM2 + Gcat ----
            # 2-stage software pipeline: stage1(b) = gather + edge matmuls,
            # stage2(b) = normalize + elu + GEMM2 + Gcat row; stage2(b) is
            # emitted after stage1(b+1) so in-order engines overlap blocks.
            FEAT = CH1 * F_IN           # hs feature region size
            with tc.tile_pool(name="p3", bufs=3) as p3, \
                 tc.tile_pool(name="pg3", bufs=1) as pg3, \
                 tc.tile_pool(name="p3s", bufs=2) as p3s, \
                 tc.tile_pool(name="psA", bufs=2, space="PSUM") as psA, \
                 tc.tile_pool(name="psB", bufs=2, space="PSUM") as psB, \
                 tc.tile_pool(name="psuv", bufs=1, space="PSUM") as psuv, \
                 tc.tile_pool(name="pst", bufs=1, space="PSUM") as pst, \
                 tc.tile_pool(name="ps2", bufs=1, space="PSUM") as ps2:
                pAB = {}

                def stage1(b):
                    sbst = p3s.tile([128, 2 * CH1 * 128], mybir.dt.int8,
                                    tag="sbst")
                    nc.sync.dma_start(sbst[:], SB1[b, :, :])
                    sb = sbst[:, 0:CH1 * 128].bitcast(fp8)
                    stb = sbst[:, CH1 * 128:2 * CH1 * 128].bitcast(fp8)
                    gl = []
                    for s in (0, 1):
                        g = pg3.tile([128, CPS * HROW], bf16, tag=f"g{s}{b % 3}")
                        gv = g[:].rearrange("p (c w) -> p c w", w=HROW)
                        if b < 3:
                            nc.vector.memset(g[:], 0.0)
                        nvs = int(nvL1[b, s])
                        if nvs > 0:
                            nc.gpsimd.dma_gather(
                                out_ap=gv[:, :, :],
                                in_ap=(Hcat_g[0:S0TOT, :] if s == 0 else
                                       Hcat_g[S0TOT:NGLOB, :]),
                                idxs_ap=idx1t[:, (b * 2 + s) * WCI:
                                              (b * 2 + s + 1) * WCI],
                                num_idxs=SIDE, num_idxs_reg=nvregs[nvs],
                                elem_size=HROW, queue_num=(b * 2 + s) % 4)
                        gl.append(g)
                    puv = psuv.tile([128, CH1 * 16], f32)
                    for c in range(CH1):
                        nc.tensor.matmul(puv[:, c * 16:(c + 1) * 16],
                                         stb[:, c * 128:(c + 1) * 128],
                                         dt_all[:, b * 16:(b + 1) * 16],
                                         start=True, stop=True)
                    uvb = p3.tile([128, CH1 * 16], bf16, tag="uvb")
                    nc.scalar.activation(uvb[:], puv[:], AF.Copy)
                    pA = psA.tile([128, F_IN], f32)
                    pB = psB.tile([128, 8], f32)
                    # hs: features [0:FEAT] chunk-major, exb tail [FEAT:FEAT+96]
                    hs = p3.tile([128, FEAT + CH1 * 8], bf16, tag="hs")
                    uvs = uvb[:].rearrange("p (c w) -> p c w", w=16)
                    for s in (0, 1):
                        g3 = gl[s][:].rearrange("p (c w) -> p c w", w=HROW)
                        tU = p3.tile([128, CPS * 8], f32, tag="tU")
                        tV = p3.tile([128, CPS * 8], f32, tag="tV")
                        exb = hs[:, FEAT + s * CPS * 8:FEAT + (s + 1) * CPS * 8]
                        nc.vector.tensor_tensor(
                            out=tU[:].rearrange("p (c w) -> p c w", w=8),
                            in0=g3[:, :, 512:520],
                            in1=uvs[:, s * CPS:(s + 1) * CPS, 0:8],
                            op=OP.mult)
                        nc.vector.tensor_tensor(
                            out=tV[:].rearrange("p (c w) -> p c w", w=8),
                            in0=g3[:, :, 520:528],
                            in1=uvs[:, s * CPS:(s + 1) * CPS, 8:16],
                            op=OP.mult)
                        nc.vector.tensor_tensor(out=exb, in0=tU[:], in1=tV[:],
                                                op=OP.max)
                        nc.vector.tensor_tensor(
                            out=hs[:, s * CPS * F_IN:(s + 1) * CPS * F_IN]
                            .rearrange("p (c h w) -> p c h w", h=HEADS, w=HID),
                            in0=g3[:, :, 0:F_IN].rearrange(
                                "p c (h w) -> p c h w", h=HEADS),
                            in1=exb.rearrange("p (c h o) -> p c h o",
                                              h=HEADS, o=1).to_broadcast(
                                [128, CPS, HEADS, HID]),
                            op=OP.mult)
                    for c in range(CH1):
                        nc.tensor.matmul(pA[:], sb[:, c * 128:(c + 1) * 128],
                                         hs[:, c * F_IN:(c + 1) * F_IN],
                                         start=(c == 0), stop=(c == CH1 - 1))
                        nc.tensor.matmul(pB[:], sb[:, c * 128:(c + 1) * 128],
                                         hs[:, FEAT + c * 8:FEAT + (c + 1) * 8],
                                         start=(c == 0), stop=(c == CH1 - 1))
                    pAB[b] = (pA, pB)

                def stage2(b):
                    pA, pB = pAB.pop(b)
                    sden = p3.tile([128, 8], f32, tag="sden")
                    nc.vector.tensor_scalar_add(sden[:], pB[:], EPS)
                    rec = p3.tile([128, 8], f32, tag="rec")
                    nc.vector.reciprocal(rec[:], sden[:])
                    o1 = p3.tile([128, F_IN], f32, tag="o1")
                    nc.vector.tensor_tensor(
                        out=o1[:].rearrange("p (h w) -> p h w", h=HEADS),
                        in0=pA[:].rearrange("p (h w) -> p h w", h=HEADS),
                        in1=rec[:].rearrange("p (h o) -> p h o", o=1).to_broadcast(
                            [128, HEADS, HID]),
                        op=OP.mult)
                    rneg = p3.tile([128, F_IN], f32, tag="rneg")
                    nc.scalar.activation(rneg[:], o1[:], AF.Relu, scale=-1.0)
                    eneg = p3.tile([128, F_IN], f32, tag="eneg")
                    nc.scalar.activation(eneg[:], rneg[:], AF.Exp, scale=-1.0)
                    m1 = p3.tile([128, F_IN], f32, tag="m1")
                    nc.vector.scalar_tensor_tensor(
                        out=m1[:], in0=o1[:], scalar=1.0, in1=eneg[:],
                        op0=OP.add, op1=OP.max)
                    h1b = p3.tile([128, F_IN], bf16, tag="h1b")
                    nc.vector.tensor_scalar_add(h1b[:], m1[:], -1.0)
                    p2t = ps2.tile([128, 69], f32)
                    for k in range(4):
                        ptt = pst.tile([128, 128], bf16)
                        nc.tensor.transpose(ptt[:], h1b[:, k * 128:(k + 1) * 128],
                                            ident[:])
                        h1tk = p3.tile([128, 128], bf16, tag="h1tk")
                        nc.scalar.activation(h1tk[:], ptt[:], AF.Copy)
                        nc.tensor.matmul(p2t[:], h1tk[:],
                                         w2t[:, k * 69:(k + 1) * 69],
                                         start=(k == 0), stop=False)
                    nc.tensor.matmul(p2t[:], e0ones[:], w2bt[:],
                                     start=False, stop=True)
                    gt = p3.tile([128, GROW], bf16, tag="gt")
                    nc.scalar.activation(gt[:, 0:64], p2t[:, 0:64], AF.Copy)
                    nc.scalar.activation(gt[:, 64:65], p2t[:, 64:65], AF.Exp)
                    nc.scalar.activation(gt[:, 65:66], p2t[:, 64:65], AF.Exp,
                                         scale=NEG_SLOPE)
                    nc.scalar.activation(gt[:, 66:67], p2t[:, 65:66], AF.Exp)
                    nc.scalar.activation(gt[:, 67:68], p2t[:, 65:66], AF.Exp,
                                         scale=NEG_SLOPE)
                    nc.scalar.activation(gt[:, 68:69], p2t[:, 68:69], AF.Copy)
                    if b < 3:
                        nc.vector.memset(gt[:, 69:GROW], 0.0)
                    nc.vector.tensor_copy(dt2_all[:, b * 2:(b + 1) * 2],
                                          gt[:, 66:68])
                    nc.scalar.dma_start(Gcat_loc[b * 128:(b + 1) * 128, :], gt[:])
                    if b == AGS0 - 1:
                        nc.gpsimd.collective_compute(
                            "AllGather", mybir.AluOpType.bypass,
                            replica_groups=groups,
                            ins=[Gcat_loc[0:R0, :]],
                            outs=[Gcat_g[0:S0TOT, :]])

                for b in range(NB + 1):
                    if b < NB:
                        stage1(b)
                    if b >= 1:
                        stage2(b - 1)

            # ---- phase 4: AllGather Gcat (tail chunk) ----
            nc.gpsimd.collective_compute(
                "AllGather", mybir.AluOpType.bypass, replica_groups=groups,
                ins=[Gcat_loc[R0:NLOC, :]],
                outs=[Gcat_g[S0TOT:NGLOB, :]])

            # ---- phase 5: L2 edges + log_softmax (2-stage pipeline) ----
            with tc.tile_pool(name="p5", bufs=3) as p5, \
                 tc.tile_pool(name="pg5", bufs=1) as pg5, \
                 tc.tile_pool(name="p5s", bufs=2) as p5s, \
                 tc.tile_pool(name="psuv2", bufs=1, space="PSUM") as psuv2, \
                 tc.tile_pool(name="ps2b", bufs=2, space="PSUM") as ps2b:
                p2of = {}

                def stage5a(b):
                    sbst5 = p5s.tile([128, 2 * CH1 * 128], mybir.dt.int8,
                                     tag="sbst5")
                    nc.sync.dma_start(sbst5[:], SB1[b, :, :])
                    sb5 = sbst5[:, 0:CH1 * 128].bitcast(fp8)
                    st5 = sbst5[:, CH1 * 128:2 * CH1 * 128].bitcast(fp8)
                    g4 = pg5.tile([128, CH1 * GROW], bf16, tag=f"g4{b % 3}")
                    g4r = g4[:].rearrange("p (c w) -> p c w", w=GROW)
                    if b < 3:
                        nc.vector.memset(g4[:], 0.0)
                    for s in (0, 1):
                        nvs = int(nvL1[b, s])
                        if nvs > 0:
                            nc.gpsimd.dma_gather(
                                out_ap=g4r[:, s * CPS:(s + 1) * CPS, :],
                                in_ap=(Gcat_g[0:S0TOT, :] if s == 0 else
                                       Gcat_g[S0TOT:NGLOB, :]),
                                idxs_ap=idx1t[:, (b * 2 + s) * WCI:
                                              (b * 2 + s + 1) * WCI],
                                num_idxs=SIDE, num_idxs_reg=nvregs[nvs],
                                elem_size=GROW, queue_num=(b * 2 + s) % 4)
                    puv2 = psuv2.tile([128, CH1 * 2], f32)
                    for c in range(CH1):
                        nc.tensor.matmul(puv2[:, c * 2:(c + 1) * 2],
                                         st5[:, c * 128:(c + 1) * 128],
                                         dt2_all[:, b * 2:(b + 1) * 2],
                                         start=True, stop=True)
                    uv2b = p5.tile([128, CH1 * 2], bf16, tag="uv2b")
                    nc.scalar.activation(uv2b[:], puv2[:], AF.Copy)
                    uv2v = uv2b[:].rearrange("p (c w) -> p c w", w=2)
                    U2 = p5.tile([128, CH1], f32, tag="U2")
                    V2 = p5.tile([128, CH1], f32, tag="V2")
                    nc.vector.tensor_tensor(
                        out=U2[:].rearrange("p (c o) -> p c o", o=1),
                        in0=g4r[:, :, 64:65], in1=uv2v[:, :, 0:1], op=OP.mult)
                    nc.vector.tensor_tensor(
                        out=V2[:].rearrange("p (c o) -> p c o", o=1),
                        in0=g4r[:, :, 65:66], in1=uv2v[:, :, 1:2], op=OP.mult)
                    ex2 = p5.tile([128, CH1], bf16, tag="ex2")
                    nc.vector.tensor_tensor(out=ex2[:], in0=U2[:], in1=V2[:],
                                            op=OP.max)
                    g4f = p5.tile([128, CH1 * GROW], bf16, tag="g4f")
                    nc.vector.tensor_tensor(
                        out=g4f[:].rearrange("p (c w) -> p c w", w=GROW),
                        in0=g4r,
                        in1=ex2[:].rearrange("p (c o) -> p c o", o=1).to_broadcast(
                            [128, CH1, GROW]),
                        op=OP.mult)
                    g4f3 = g4f[:].rearrange("p (c w) -> p c w", w=GROW)
                    p2b = ps2b.tile([128, 69], f32)
                    for c in range(CH1):
                        nc.tensor.matmul(p2b[:], sb5[:, c * 128:(c + 1) * 128],
                                         g4f3[:, c, 0:69],
                                         start=(c == 0), stop=(c == CH1 - 1))
                    p2of[b] = p2b

                def stage5b(b):
                    p2b = p2of.pop(b)
                    den2 = p5.tile([128, 1], f32, tag="den2")
                    nc.vector.tensor_scalar_add(den2[:], p2b[:, 68:69], EPS)
                    rec2 = p5.tile([128, 1], f32, tag="rec2")
                    nc.vector.reciprocal(rec2[:], den2[:])
                    ozt = p5.tile([128, 128], f32, tag="ozt")
                    Zt = p5.tile([128, 64], f32, tag="Zt")
                    nc.vector.tensor_scalar_mul(Zt[:], p2b[:, 0:64], rec2[:, 0:1])
                    nc.scalar.activation(ozt[:, 64:128], Zt[:], AF.Copy)
                    mx = p5.tile([128, 1], f32, tag="mx")
                    nc.vector.reduce_max(mx[:], Zt[:], axis=mybir.AxisListType.X)
                    nmx = p5.tile([128, 1], f32, tag="nmx")
                    nc.vector.tensor_scalar_mul(nmx[:], mx[:], -1.0)
                    ez = p5.tile([128, 64], f32, tag="ez")
                    nc.scalar.activation(ez[:], Zt[:], AF.Exp, bias=nmx[:, 0:1])
                    sz = p5.tile([128, 1], f32, tag="sz")
                    nc.vector.reduce_sum(sz[:], ez[:], axis=mybir.AxisListType.X)
                    lse = p5.tile([128, 1], f32, tag="lse")
                    nc.scalar.activation(lse[:], sz[:], AF.Ln)
                    nc.vector.tensor_scalar(
                        out=ozt[:, 0:64], in0=Zt[:], scalar1=mx[:, 0:1],
                        scalar2=lse[:, 0:1], op0=OP.subtract, op1=OP.subtract)
                    nc.scalar.dma_start(out_cat[b * 128:(b + 1) * 128, :], ozt[:])

                for b in range(NB + 1):
                    if b < NB:
                        stage5a(b)
                    if b >= 1:
                        stage5b(b - 1)

    mybir.codegen_inst_isa_subclasses(nc)
    return nc


# ---------------- top-level entry ----------------

_CACHE = {}


def kernel(x, edge_index, W1, att_src1, att_dst1, b1, W2, att_src2, att_dst2,
           b2, _trace=False):
    in_maps, meta = _host_prep(x, edge_index, W1, att_src1, att_dst1, b1, W2,
                               att_src2, att_dst2, b2)
    if "prog" not in _CACHE:
        _CACHE["prog"] = _build_program(meta["nvL1"])
    nc = _CACHE["prog"]
    res = run_bass_kernel_spmd(nc, in_maps, list(range(NCORES)), trace=_trace)
    node_of_gid = meta["node_of_gid"]
    out = np.zeros((N, 64), np.float32)
    Z = np.zeros((N, 64), np.float32)
    for c in range(NCORES):
        oc = np.asarray(res.results[c]["out_cat"])
        cols = node_of_gid[c * NLOC:(c + 1) * NLOC]
        valid = cols >= 0
        out[cols[valid]] = oc[valid, 0:64]
        Z[cols[valid]] = oc[valid, 64:128]
    kernel._last_exec_ns = res.exec_time_ns
    kernel._last_res = res
    return (out, Z)


# revision 14
# speedup vs baseline: 4.0218x; 1.0323x over previous
"""GAT 2-layer kernel for 8 trn2 NeuronCores (self-contained).

Sharding: destination-node blocks across 8 cores. Per core: 49 blocks x 128
nodes. Layer GEMMs computed on owned nodes; per-node attention factors
(u=exp(a_src), v=exp(0.2 a_src), and dst-side) folded into gatherable row
tables (Hcat/Gcat) that are AllGathered; edge phases gather rows by src via
dma_gather and do segment-softmax-aggregation with per-chunk selection-matrix
matmuls (exp(leaky(x)) == max(exp(x), exp(0.2 x)) makes the logits separable).
Edge slots are split by source half (int16 gather indices); both layers share
the same slot assignment so one selection-matrix pair and one index table
serve both edge phases.
"""
import sys, types
sys.path.insert(0, '/opt/trn_rl_repo')

import numpy as np
import ml_dtypes

# ---------------- problem constants (hardcoded) ----------------
N = 50000
F_IN = 512
HID = 64
HEADS = 8
CLS = 64
NEG_SLOPE = 0.2
NCORES = 8
NPC = 6250
NB = 49
BLK = 128
NLOC = NB * BLK           # 6272
NGLOB = NCORES * NLOC     # 50176
CPS = 6                   # chunks per side per block
SIDE = CPS * 128          # 768 edge slots per side
CH1 = 2 * CPS             # 12 chunks per block
WCI = SIDE // 16          # 48 idx cols per side
HROW = 640                # Hcat row (bf16)
GROW = 128                # Gcat row (bf16)
AGS0 = 25                 # blocks per core in side 0
R0 = AGS0 * BLK           # 3200 rows per core in side 0
R1 = NLOC - R0            # 3072 rows per core in side 1
S0TOT = NCORES * R0       # 25600 rows in side 0
S1TOT = NCORES * R1       # 24576 rows in side 1
HCH = [(0, 13), (13, 25), (25, 37), (37, 49)]  # AG chunks (block ranges)
HBASE = []                # global row base of each AG chunk
_acc = 0
for _lo, _hi in HCH:
    HBASE.append(_acc)
    _acc += NCORES * (_hi - _lo) * BLK
EPS = 1e-16


def _install_ntff_hook():
    if 'antenv.axon_hooks' in sys.modules:
        return
    try:
        sys.path.insert(0, '/root/.axon_site')
        from trn_agent_boot.trn_boot import _ntff_profile_via_ctypes
        hook = _ntff_profile_via_ctypes('/opt/axon/libaxon_pjrt.so')
    except Exception:
        return
    mod = types.ModuleType('antenv.axon_hooks')
    mod._hook = hook
    mod.get_axon_ntff_profile_hook = lambda: mod._hook
    mod.set_axon_ntff_profile_hook = lambda h: setattr(mod, '_hook', h)
    sys.modules['antenv.axon_hooks'] = mod


_install_ntff_hook()

import concourse.bass as bass
import concourse.mybir as mybir
import concourse.tile as tile
from concourse import library_config
from concourse.bass_utils import run_bass_kernel_spmd
from concourse.vector_clock import VectorClock, ScopedClock

bf16 = mybir.dt.bfloat16
f32 = mybir.dt.float32
fp8 = mybir.dt.float8e4

# ------------- tile framework patches (walrus: 1 sync wait / inst) ---------


def _drain_and_barrier_split(self, tick_clock, wait_clock):
    nc = self.nc
    full = tick_clock.global_clock
    procs = [p for p in range(27) if full[p] > 0]
    for p in procs:
        sub = VectorClock([full[q] if q == p else 0 for q in range(27)])
        drain_inst = nc.sync.drain(fusable=False)
        wait_clock.add_sem_waits(drain_inst.ins, ScopedClock({None: sub}))
    if not procs:
        nc.sync.drain(fusable=False)
    nc.all_engine_barrier()
    assert self.sems is not None
    popped = nc._tile_sem_poison_stack.pop()
    assert popped is self._sem_poison
    nc.clear_and_free_semaphores(list(self.sems.allocated().values()))
    nc.all_engine_barrier()


def _split_excess_waits(nc):
    for bb in nc.main_func.blocks:
        insts = bb.instructions
        i = 0
        while i < len(insts):
            ins = insts[i]
            si = ins.sync_info
            if si is None:
                i += 1
                continue
            waits = list(si.on_wait)
            if len(waits) <= 1:
                i += 1
                continue
            keep, surplus = waits[:1], waits[1:]
            ins.sync_info = mybir.SyncInfo(on_wait=keep, on_update=list(si.on_update))
            nops = []
            for w in surplus:
                nop = mybir.InstNoOp(name=nc.get_next_instruction_name())
                nop.engine = ins.engine
                nop.sync_info = mybir.SyncInfo(on_wait=[w], on_update=[])
                nc.register_instruction(nop)
                nops.append(nop)
            for k, nop in enumerate(nops):
                insts.insert(i + k, nop)
            i += 1 + len(nops)


_PATCHED = False


def _install_patches():
    global _PATCHED
    if _PATCHED:
        return
    _orig_exit = tile.TileContext.__exit__

    def _exit_with_split(self, *a):
        r = _orig_exit(self, *a)
        _split_excess_waits(self.nc)
        return r

    tile.TileContext._drain_and_barrier = _drain_and_barrier_split
    tile.TileContext.__exit__ = _exit_with_split
    _PATCHED = True


# ---------------- host-side graph preprocessing ----------------


def _wrap16(flat_idx):
    W = len(flat_idx) // 16
    t = np.asarray(flat_idx, np.int16).reshape(W, 16).T
    return np.tile(t, (8, 1))


def _host_prep(x, edge_index, W1, att_src1, att_dst1, b1, W2, att_src2,
               att_dst2, b2):
    src_o = np.concatenate([np.asarray(edge_index[0]),
                            np.arange(N, dtype=np.int64)]).astype(np.int64)
    dst_o = np.concatenate([np.asarray(edge_index[1]),
                            np.arange(N, dtype=np.int64)]).astype(np.int64)

    core_of = dst_o // NPC
    deg = np.bincount(dst_o, minlength=N)

    gid = np.full(N, -1, np.int64)
    node_of_gid = np.full(NGLOB, -1, np.int64)
    for c in range(NCORES):
        nodes = np.arange(c * NPC, (c + 1) * NPC)
        d = deg[nodes]
        order = np.argsort(-d, kind='stable')
        blk_cnt = np.zeros(NB, np.int64)
        blk_load = np.zeros(NB, np.int64)
        assign = np.full(NPC, -1, np.int64)
        for i in order:
            b = int(np.argmin(blk_load + (blk_cnt >= BLK) * (1 << 40)))
            assign[i] = b
            blk_cnt[b] += 1
            blk_load[b] += d[i]
        slot_ctr = np.zeros(NB, np.int64)
        for i in range(NPC):
            b = assign[i]
            g = c * NLOC + b * BLK + slot_ctr[b]
            slot_ctr[b] += 1
            gid[nodes[i]] = g
            node_of_gid[g] = nodes[i]

    src_g = gid[src_o]
    dst_g = gid[dst_o]
    dst_block = (dst_g % NLOC) // BLK
    dst_slot = dst_g % BLK
    src_core = src_g // NLOC
    src_row = src_g % NLOC
    src_blk = src_row // BLK
    side = (src_row >= R0).astype(np.int64)
    src_gidx = np.zeros_like(src_g)
    for _r, (_lo, _hi) in enumerate(HCH):
        _m = (src_blk >= _lo) & (src_blk < _hi)
        _sz = (_hi - _lo) * BLK
        src_gidx[_m] = (HBASE[_r] + src_core[_m] * _sz
                        + (src_row[_m] - _lo * BLK))
    src_sidx = np.where(side == 0, src_gidx, src_gidx - S0TOT)

    idxL1 = np.full((NCORES, NB, 2, SIDE), -1, np.int32)
    dslL1 = np.full((NCORES, NB, 2, SIDE), -1, np.int32)
    cntL1 = np.zeros((NCORES, NB, 2), np.int64)

    for c in range(NCORES):
        em = np.nonzero(core_of == c)[0]
        eb = dst_block[em]
        for b in range(NB):
            eidx = em[eb == b]
            dslots = dst_slot[eidx]
            sides = side[eidx]
            for s in (0, 1):
                ms = sides == s
                k = int(ms.sum())
                if k > SIDE:
                    raise RuntimeError(f"L1 overflow c{c} b{b} s{s}: {k}")
                idxL1[c, b, s, :k] = src_sidx[eidx][ms]
                dslL1[c, b, s, :k] = dslots[ms]
                cntL1[c, b, s] = k

    # SPMD: same valid-descriptor count on every core -> pad with idx 0
    nvmax = cntL1.max(axis=0)      # [NB, 2]
    nvL1 = np.minimum(((nvmax + 63) // 64) * 64, SIDE)
    for c in range(NCORES):
        for b in range(NB):
            for s in (0, 1):
                k = int(cntL1[c, b, s])
                idxL1[c, b, s, k:int(nvL1[b, s])] = 0

    SL1 = np.zeros((NCORES, NB, 128, CH1 * 128), np.float32)
    STL1 = np.zeros((NCORES, NB, 128, CH1 * 128), np.float32)
    for c in range(NCORES):
        for b in range(NB):
            for s in (0, 1):
                for j in range(CPS):
                    ch = s * CPS + j
                    sl = dslL1[c, b, s, j * 128:(j + 1) * 128]
                    e_i = np.nonzero(sl >= 0)[0]
                    n_i = sl[e_i]
                    SL1[c, b, e_i, ch * 128 + n_i] = 1
                    STL1[c, b, n_i, ch * 128 + e_i] = 1
    SB1 = np.concatenate([SL1, STL1], axis=-1).astype(
        ml_dtypes.float8_e4m3).view(np.int8)  # [NCORES, NB, 128, 2*CH1*128]

    W1 = np.asarray(W1, np.float32)
    att_src1 = np.asarray(att_src1, np.float32)
    att_dst1 = np.asarray(att_dst1, np.float32)
    b1 = np.asarray(b1, np.float32)
    W2 = np.asarray(W2, np.float32)
    att_src2 = np.asarray(att_src2, np.float32)
    att_dst2 = np.asarray(att_dst2, np.float32)
    b2 = np.asarray(b2, np.float32)

    Msrc = np.zeros((F_IN, HEADS), np.float32)
    Mdst = np.zeros((F_IN, HEADS), np.float32)
    for h in range(HEADS):
        Msrc[h * HID:(h + 1) * HID, h] = att_src1[h]
        Mdst[h * HID:(h + 1) * HID, h] = att_dst1[h]
    # h features stored head-major (natural W1 layout: col = head*64 + ch)
    W1aug = np.zeros((F_IN + 128, F_IN + 16), np.float32)
    W1aug[:F_IN, 0:F_IN] = W1
    W1aug[:F_IN, F_IN:F_IN + 8] = W1 @ Msrc
    W1aug[:F_IN, F_IN + 8:F_IN + 16] = W1 @ Mdst
    W1aug[F_IN, 0:F_IN] = b1
    W2aug = np.zeros((F_IN, 69), np.float32)
    W2aug[:, 0:CLS] = W2
    W2aug[:, 64] = W2 @ att_src2[0]
    W2aug[:, 65] = W2 @ att_dst2[0]
    W2bias = np.zeros((128, 69), np.float32)
    W2bias[0, 0:CLS] = b2
    W2bias[0, 68] = 1.0

    x = np.asarray(x, np.float32)
    xTs = []
    for c in range(NCORES):
        cols = node_of_gid[c * NLOC:(c + 1) * NLOC]
        validc = cols >= 0
        xc = np.zeros((NLOC, F_IN), np.float32)
        xc[validc] = x[cols[validc]]
        xt = np.zeros((F_IN + 128, NLOC), np.float32)
        xt[:F_IN] = xc.T
        xt[F_IN] = 1.0
        xTs.append(xt.astype(ml_dtypes.bfloat16))

    idxL1_dev = np.zeros((NCORES, 128, NB * 2 * WCI), np.int16)
    for c in range(NCORES):
        for b in range(NB):
            for s in (0, 1):
                col = (b * 2 + s) * WCI
                idxL1_dev[c, :, col:col + WCI] = _wrap16(idxL1[c, b, s, :])

    in_maps = []
    W1aug_bf = W1aug.astype(ml_dtypes.bfloat16)
    W2aug_bf = W2aug.astype(ml_dtypes.bfloat16)
    W2bias_bf = W2bias.astype(ml_dtypes.bfloat16)
    for c in range(NCORES):
        in_maps.append({
            "xT": np.ascontiguousarray(xTs[c]),
            "W1aug": W1aug_bf,
            "W2aug": W2aug_bf,
            "W2bias": W2bias_bf,
            "idxL1": np.ascontiguousarray(idxL1_dev[c]),
            "SB1": np.ascontiguousarray(SB1[c]),
        })
    meta = {"node_of_gid": node_of_gid, "nvL1": nvL1}
    return in_maps, meta


# ---------------- device program ----------------


def _build_program(nvL1):
    _install_patches()
    nc = bass.Bass(num_swdge_queues=4)
    AF = mybir.ActivationFunctionType
    OP = mybir.AluOpType
    KW = (F_IN + 128) // 128          # 5 k-chunks for GEMM1
    WROW = F_IN + 16                  # 528 W1aug cols

    xT = nc.dram_tensor("xT", [F_IN + 128, NLOC], bf16, kind="ExternalInput")
    W1a = nc.dram_tensor("W1aug", [F_IN + 128, WROW], bf16, kind="ExternalInput")
    W2a = nc.dram_tensor("W2aug", [F_IN, 69], bf16, kind="ExternalInput")
    W2b = nc.dram_tensor("W2bias", [128, 69], bf16, kind="ExternalInput")
    idxL1 = nc.dram_tensor("idxL1", [128, NB * 2 * WCI], mybir.dt.int16,
                           kind="ExternalInput")
    SB1 = nc.dram_tensor("SB1", [NB, 128, 2 * CH1 * 128],
                         mybir.dt.int8, kind="ExternalInput")

    out_cat = nc.dram_tensor("out_cat", [NLOC, 128], f32, kind="ExternalOutput")

    Hcat_loc = nc.dram_tensor("Hcat_loc", [NLOC, HROW], bf16)
    Hcat_g = nc.dram_tensor("Hcat_g", [NGLOB, HROW], bf16, addr_space="Shared")
    Gcat_loc = nc.dram_tensor("Gcat_loc", [NLOC, GROW], bf16)
    Gcat_g = nc.dram_tensor("Gcat_g", [NGLOB, GROW], bf16, addr_space="Shared")

    groups = [list(range(NCORES))]

    with tile.TileContext(nc) as tc:
        with tc.tile_critical():
            nc.gpsimd.load_library(library_config.mlp)
        nvset = sorted({int(v) for v in nvL1.flatten()})
        nvregs = {}
        for v in nvset:
            if v > 0:
                nvregs[v] = nc.gpsimd.to_reg(v)

        with tc.tile_pool(name="const", bufs=1) as constp:
            w1t = constp.tile([128, KW * WROW], bf16)
            for k in range(KW):
                nc.sync.dma_start(w1t[:, k * WROW:(k + 1) * WROW],
                                  W1a[k * 128:(k + 1) * 128, :])
            w2t = constp.tile([128, 4 * 69], bf16)
            for k in range(4):
                nc.sync.dma_start(w2t[:, k * 69:(k + 1) * 69],
                                  W2a[k * 128:(k + 1) * 128, :])
            w2bt = constp.tile([128, 69], bf16)
            nc.sync.dma_start(w2bt[:], W2b[:, :])
            e0ones = constp.tile([128, 128], bf16)
            nc.vector.memset(e0ones[:], 0.0)
            nc.vector.memset(e0ones[0:1, :], 1.0)
            ident = constp.tile([128, 128], bf16)
            from concourse.masks import make_identity
            make_identity(nc, ident[:])
            dt_all = constp.tile([128, NB * 16], bf16)
            dt2_all = constp.tile([128, NB * 2], bf16)
            idx1t = constp.tile([128, NB * 2 * WCI], mybir.dt.int16)
            nc.sync.dma_start(idx1t[:], idxL1[:, :])

            # ---- phase 1: GEMM1 + Hcat rows ----
            with tc.tile_pool(name="p1", bufs=3) as p1, \
                 tc.tile_pool(name="ps1a", bufs=2, space="PSUM") as ps1a, \
                 tc.tile_pool(name="ps1b", bufs=2, space="PSUM") as ps1b:
                for b in range(NB):
                    pA = ps1a.tile([128, F_IN], f32)
                    pB = ps1b.tile([128, 16], f32)
                    xt = p1.tile([128, KW * 128], bf16, tag="xt")
                    nc.sync.dma_start(
                        xt[:].rearrange("p (k j) -> p k j", k=KW),
                        xT[0:KW * 128, b * 128:(b + 1) * 128].rearrange(
                            "(k p) j -> p k j", k=KW))
                    for k in range(KW):
                        nc.tensor.matmul(pA[:], xt[:, k * 128:(k + 1) * 128],
                                         w1t[:, k * WROW:k * WROW + F_IN],
                                         start=(k == 0), stop=(k == KW - 1))
                        nc.tensor.matmul(pB[:], xt[:, k * 128:(k + 1) * 128],
                                         w1t[:, k * WROW + F_IN:(k + 1) * WROW],
                                         start=(k == 0), stop=(k == KW - 1))
                    hc = p1.tile([128, HROW], bf16, tag="hc")
                    nc.scalar.activation(hc[:, 0:F_IN], pA[:], AF.Copy)
                    nc.scalar.activation(hc[:, 512:520], pB[:, 0:8], AF.Exp)
                    nc.scalar.activation(hc[:, 520:528], pB[:, 0:8], AF.Exp,
                                         scale=NEG_SLOPE)
                    nc.scalar.activation(hc[:, 528:536], pB[:, 8:16], AF.Exp)
                    nc.scalar.activation(hc[:, 536:544], pB[:, 8:16], AF.Exp,
                                         scale=NEG_SLOPE)
                    if b < 3:
                        nc.vector.memset(hc[:, 544:HROW], 0.0)
                    nc.vector.tensor_copy(dt_all[:, b * 16:(b + 1) * 16],
                                          hc[:, 528:544])
                    nc.scalar.dma_start(Hcat_loc[b * 128:(b + 1) * 128, :], hc[:])
                    for r, (lo, hi) in enumerate(HCH[:-1]):
                        if b == hi - 1:
                            nc.gpsimd.collective_compute(
                                "AllGather", mybir.AluOpType.bypass,
                                replica_groups=groups,
                                ins=[Hcat_loc[lo * BLK:hi * BLK, :]],
                                outs=[Hcat_g[HBASE[r]:HBASE[r] + NCORES
                                             * (hi - lo) * BLK, :]])

            # ---- phase 2: AllGather Hcat (tail chunk) ----
            lo, hi = HCH[-1]
            nc.gpsimd.collective_compute(
                "AllGather", mybir.AluOpType.bypass, replica_groups=groups,
                ins=[Hcat_loc[lo * BLK:hi * BLK, :]],
                outs=[Hcat_g[HBASE[-1]:NGLOB, :]])

            # ---- phase 3: L1 edges + block tails + GEMM2 + Gcat ----
            # 2-stage software pipeline: stage1(b) = gather + edge matmuls,
            # stage2(b) = normalize + elu + GEMM2 + Gcat row; stage2(b) is
            # emitted after stage1(b+1) so in-order engines overlap blocks.
            FEAT = CH1 * F_IN           # hs feature region size
            with tc.tile_pool(name="p3", bufs=3) as p3, \
                 tc.tile_pool(name="pg3", bufs=1) as pg3, \
                 tc.tile_pool(name="p3s", bufs=2) as p3s, \
                 tc.tile_pool(name="psA", bufs=2, space="PSUM") as psA, \
                 tc.tile_pool(name="psB", bufs=2, space="PSUM") as psB, \
                 tc.tile_pool(name="psuv", bufs=1, space="PSUM") as psuv, \
                 tc.tile_pool(name="pst", bufs=1, space="PSUM") as pst, \
                 tc.tile_pool(name="ps2", bufs=1, space="PSUM") as ps2:
                pAB = {}

                def stage1(b):
                    sbst = p3s.tile([128, 2 * CH1 * 128], mybir.dt.int8,
                                    tag="sbst")
                    nc.sync.dma_start(sbst[:], SB1[b, :, :])
                    sb = sbst[:, 0:CH1 * 128].bitcast(fp8)
                    stb = sbst[:, CH1 * 128:2 * CH1 * 128].bitcast(fp8)
                    gl = []
                    for s in (0, 1):
                        g = pg3.tile([128, CPS * HROW], bf16, tag=f"g{s}{b % 3}")
                        gv = g[:].rearrange("p (c w) -> p c w", w=HROW)
                        if b < 3:
                            nc.vector.memset(g[:], 0.0)
                        nvs = int(nvL1[b, s])
                        if nvs > 0:
                            nc.gpsimd.dma_gather(
                                out_ap=gv[:, :, :],
                                in_ap=(Hcat_g[0:S0TOT, :] if s == 0 else
                                       Hcat_g[S0TOT:NGLOB, :]),
                                idxs_ap=idx1t[:, (b * 2 + s) * WCI:
                                              (b * 2 + s + 1) * WCI],
                                num_idxs=SIDE, num_idxs_reg=nvregs[nvs],
                                elem_size=HROW, queue_num=(b * 2 + s) % 4)
                        gl.append(g)
                    puv = psuv.tile([128, CH1 * 16], f32)
                    for c in range(CH1):
                        nc.tensor.matmul(puv[:, c * 16:(c + 1) * 16],
                                         stb[:, c * 128:(c + 1) * 128],
                                         dt_all[:, b * 16:(b + 1) * 16],
                                         start=True, stop=True)
                    uvb = p3.tile([128, CH1 * 16], bf16, tag="uvb")
                    nc.scalar.activation(uvb[:], puv[:], AF.Copy)
                    pA = psA.tile([128, F_IN], f32)
                    pB = psB.tile([128, 8], f32)
                    # hs: features [0:FEAT] chunk-major, exb tail [FEAT:FEAT+96]
                    hs = p3.tile([128, FEAT + CH1 * 8], bf16, tag="hs")
                    uvs = uvb[:].rearrange("p (c w) -> p c w", w=16)
                    for s in (0, 1):
                        g3 = gl[s][:].rearrange("p (c w) -> p c w", w=HROW)
                        tU = p3.tile([128, CPS * 8], f32, tag="tU")
                        tV = p3.tile([128, CPS * 8], f32, tag="tV")
                        exb = hs[:, FEAT + s * CPS * 8:FEAT + (s + 1) * CPS * 8]
                        nc.vector.tensor_tensor(
                            out=tU[:].rearrange("p (c w) -> p c w", w=8),
                            in0=g3[:, :, 512:520],
                            in1=uvs[:, s * CPS:(s + 1) * CPS, 0:8],
                            op=OP.mult)
                        nc.vector.tensor_tensor(
                            out=tV[:].rearrange("p (c w) -> p c w", w=8),
                            in0=g3[:, :, 520:528],
                            in1=uvs[:, s * CPS:(s + 1) * CPS, 8:16],
                            op=OP.mult)
                        nc.vector.tensor_tensor(out=exb, in0=tU[:], in1=tV[:],
                                                op=OP.max)
                        nc.vector.tensor_tensor(
                            out=hs[:, s * CPS * F_IN:(s + 1) * CPS * F_IN]
                            .rearrange("p (c h w) -> p c h w", h=HEADS, w=HID),
                            in0=g3[:, :, 0:F_IN].rearrange(
                                "p c (h w) -> p c h w", h=HEADS),
                            in1=exb.rearrange("p (c h o) -> p c h o",
                                              h=HEADS, o=1).to_broadcast(
                                [128, CPS, HEADS, HID]),
                            op=OP.mult)
                    for c in range(CH1):
                        nc.tensor.matmul(pA[:], sb[:, c * 128:(c + 1) * 128],
                                         hs[:, c * F_IN:(c + 1) * F_IN],
                                         start=(c == 0), stop=(c == CH1 - 1))
                        nc.tensor.matmul(pB[:], sb[:, c * 128:(c + 1) * 128],
                                         hs[:, FEAT + c * 8:FEAT + (c + 1) * 8],
                                         start=(c == 0), stop=(c == CH1 - 1))
                    pAB[b] = (pA, pB)

                def stage2(b):
                    pA, pB = pAB.pop(b)
                    sden = p3.tile([128, 8], f32, tag="sden")
                    nc.vector.tensor_scalar_add(sden[:], pB[:], EPS)
                    rec = p3.tile([128, 8], f32, tag="rec")
                    nc.vector.reciprocal(rec[:], sden[:])
                    o1 = p3.tile([128, F_IN], f32, tag="o1")
                    nc.vector.tensor_tensor(
                        out=o1[:].rearrange("p (h w) -> p h w", h=HEADS),
                        in0=pA[:].rearrange("p (h w) -> p h w", h=HEADS),
                        in1=rec[:].rearrange("p (h o) -> p h o", o=1).to_broadcast(
                            [128, HEADS, HID]),
                        op=OP.mult)
                    rneg = p3.tile([128, F_IN], f32, tag="rneg")
                    nc.scalar.activation(rneg[:], o1[:], AF.Relu, scale=-1.0)
                    eneg = p3.tile([128, F_IN], f32, tag="eneg")
                    nc.scalar.activation(eneg[:], rneg[:], AF.Exp, scale=-1.0)
                    m1 = p3.tile([128, F_IN], f32, tag="m1")
                    nc.vector.scalar_tensor_tensor(
                        out=m1[:], in0=o1[:], scalar=1.0, in1=eneg[:],
                        op0=OP.add, op1=OP.max)
                    h1b = p3.tile([128, F_IN], bf16, tag="h1b")
                    nc.vector.tensor_scalar_add(h1b[:], m1[:], -1.0)
                    p2t = ps2.tile([128, 69], f32)
                    for k in range(4):
                        ptt = pst.tile([128, 128], bf16)
                        nc.tensor.transpose(ptt[:], h1b[:, k * 128:(k + 1) * 128],
                                            ident[:])
                        h1tk = p3.tile([128, 128], bf16, tag="h1tk")
                        nc.scalar.activation(h1tk[:], ptt[:], AF.Copy)
                        nc.tensor.matmul(p2t[:], h1tk[:],
                                         w2t[:, k * 69:(k + 1) * 69],
                                         start=(k == 0), stop=False)
                    nc.tensor.matmul(p2t[:], e0ones[:], w2bt[:],
                                     start=False, stop=True)
                    gt = p3.tile([128, GROW], bf16, tag="gt")
                    nc.scalar.activation(gt[:, 0:64], p2t[:, 0:64], AF.Copy)
                    nc.scalar.activation(gt[:, 64:65], p2t[:, 64:65], AF.Exp)
                    nc.scalar.activation(gt[:, 65:66], p2t[:, 64:65], AF.Exp,
                                         scale=NEG_SLOPE)
                    nc.scalar.activation(gt[:, 66:67], p2t[:, 65:66], AF.Exp)
                    nc.scalar.activation(gt[:, 67:68], p2t[:, 65:66], AF.Exp,
                                         scale=NEG_SLOPE)
                    nc.scalar.activation(gt[:, 68:69], p2t[:, 68:69], AF.Copy)
                    if b < 3:
                        nc.vector.memset(gt[:, 69:GROW], 0.0)
                    nc.scalar.activation(dt2_all[:, b * 2:(b + 1) * 2],
                                         gt[:, 66:68], AF.Copy)
                    nc.scalar.dma_start(Gcat_loc[b * 128:(b + 1) * 128, :], gt[:])
                    for r, (lo, hi) in enumerate(HCH[:-1]):
                        if b == hi - 1:
                            nc.gpsimd.collective_compute(
                                "AllGather", mybir.AluOpType.bypass,
                                replica_groups=groups,
                                ins=[Gcat_loc[lo * BLK:hi * BLK, :]],
                                outs=[Gcat_g[HBASE[r]:HBASE[r] + NCORES
                                             * (hi - lo) * BLK, :]])

                for b in range(NB + 1):
                    if b < NB:
                        stage1(b)
                    if b >= 1:
                        stage2(b - 1)

            # ---- phase 4: AllGather Gcat (tail chunk) ----
            lo, hi = HCH[-1]
            nc.gpsimd.collective_compute(
                "AllGather", mybir.AluOpType.bypass, replica_groups=groups,
                ins=[Gcat_loc[lo * BLK:hi * BLK, :]],
                outs=[Gcat_g[HBASE[-1]:NGLOB, :]])

            # ---- phase 5: L2 edges + log_softmax (2-stage pipeline) ----
            with tc.tile_pool(name="p5", bufs=3) as p5, \
                 tc.tile_pool(name="pg5", bufs=1) as pg5, \
                 tc.tile_pool(name="p5s", bufs=2) as p5s, \
                 tc.tile_pool(name="psuv2", bufs=1, space="PSUM") as psuv2, \
                 tc.tile_pool(name="ps2b", bufs=2, space="PSUM") as ps2b:
                p2of = {}

                def stage5a(b):
                    sbst5 = p5s.tile([128, 2 * CH1 * 128], mybir.dt.int8,
                                     tag="sbst5")
                    nc.sync.dma_start(sbst5[:], SB1[b, :, :])
                    sb5 = sbst5[:, 0:CH1 * 128].bitcast(fp8)
                    st5 = sbst5[:, CH1 * 128:2 * CH1 * 128].bitcast(fp8)
                    g4 = pg5.tile([128, CH1 * GROW], bf16, tag=f"g4{b % 3}")
                    g4r = g4[:].rearrange("p (c w) -> p c w", w=GROW)
                    if b < 3:
                        nc.vector.memset(g4[:], 0.0)
                    for s in (0, 1):
                        nvs = int(nvL1[b, s])
                        if nvs > 0:
                            nc.gpsimd.dma_gather(
                                out_ap=g4r[:, s * CPS:(s + 1) * CPS, :],
                                in_ap=(Gcat_g[0:S0TOT, :] if s == 0 else
                                       Gcat_g[S0TOT:NGLOB, :]),
                                idxs_ap=idx1t[:, (b * 2 + s) * WCI:
                                              (b * 2 + s + 1) * WCI],
                                num_idxs=SIDE, num_idxs_reg=nvregs[nvs],
                                elem_size=GROW, queue_num=(b * 2 + s) % 4)
                    puv2 = psuv2.tile([128, CH1 * 2], f32)
                    for c in range(CH1):
                        nc.tensor.matmul(puv2[:, c * 2:(c + 1) * 2],
                                         st5[:, c * 128:(c + 1) * 128],
                                         dt2_all[:, b * 2:(b + 1) * 2],
                                         start=True, stop=True)
                    uv2b = p5.tile([128, CH1 * 2], bf16, tag="uv2b")
                    nc.scalar.activation(uv2b[:], puv2[:], AF.Copy)
                    uv2v = uv2b[:].rearrange("p (c w) -> p c w", w=2)
                    U2 = p5.tile([128, CH1], f32, tag="U2")
                    V2 = p5.tile([128, CH1], f32, tag="V2")
                    nc.vector.tensor_tensor(
                        out=U2[:].rearrange("p (c o) -> p c o", o=1),
                        in0=g4r[:, :, 64:65], in1=uv2v[:, :, 0:1], op=OP.mult)
                    nc.vector.tensor_tensor(
                        out=V2[:].rearrange("p (c o) -> p c o", o=1),
                        in0=g4r[:, :, 65:66], in1=uv2v[:, :, 1:2], op=OP.mult)
                    ex2 = p5.tile([128, CH1], bf16, tag="ex2")
                    nc.vector.tensor_tensor(out=ex2[:], in0=U2[:], in1=V2[:],
                                            op=OP.max)
                    g4f = p5.tile([128, CH1 * GROW], bf16, tag="g4f")
                    nc.vector.tensor_tensor(
                        out=g4f[:].rearrange("p (c w) -> p c w", w=GROW),
                        in0=g4r,
                        in1=ex2[:].rearrange("p (c o) -> p c o", o=1).to_broadcast(
                            [128, CH1, GROW]),
                        op=OP.mult)
                    g4f3 = g4f[:].rearrange("p (c w) -> p c w", w=GROW)
                    p2b = ps2b.tile([128, 69], f32)
                    for c in range(CH1):
                        nc.tensor.matmul(p2b[:], sb5[:, c * 128:(c + 1) * 128],
                                         g4f3[:, c, 0:69],
                                         start=(c == 0), stop=(c == CH1 - 1))
                    p2of[b] = p2b

                def stage5b(b):
                    p2b = p2of.pop(b)
                    den2 = p5.tile([128, 1], f32, tag="den2")
                    nc.vector.tensor_scalar_add(den2[:], p2b[:, 68:69], EPS)
                    rec2 = p5.tile([128, 1], f32, tag="rec2")
                    nc.vector.reciprocal(rec2[:], den2[:])
                    ozt = p5.tile([128, 128], f32, tag="ozt")
                    Zt = p5.tile([128, 64], f32, tag="Zt")
                    nc.vector.tensor_scalar_mul(Zt[:], p2b[:, 0:64], rec2[:, 0:1])
                    nc.scalar.activation(ozt[:, 64:128], Zt[:], AF.Copy)
                    mx = p5.tile([128, 1], f32, tag="mx")
                    nc.vector.reduce_max(mx[:], Zt[:], axis=mybir.AxisListType.X)
                    nmx = p5.tile([128, 1], f32, tag="nmx")
                    nc.vector.tensor_scalar_mul(nmx[:], mx[:], -1.0)
                    ez = p5.tile([128, 64], f32, tag="ez")
                    nc.scalar.activation(ez[:], Zt[:], AF.Exp, bias=nmx[:, 0:1])
                    sz = p5.tile([128, 1], f32, tag="sz")
                    nc.vector.reduce_sum(sz[:], ez[:], axis=mybir.AxisListType.X)
                    lse = p5.tile([128, 1], f32, tag="lse")
                    nc.scalar.activation(lse[:], sz[:], AF.Ln)
                    nc.vector.tensor_scalar(
                        out=ozt[:, 0:64], in0=Zt[:], scalar1=mx[:, 0:1],
                        scalar2=lse[:, 0:1], op0=OP.subtract, op1=OP.subtract)
                    nc.scalar.dma_start(out_cat[b * 128:(b + 1) * 128, :], ozt[:])

                for b in range(NB + 1):
                    if b < NB:
                        stage5a(b)
                    if b >= 1:
                        stage5b(b - 1)

    mybir.codegen_inst_isa_subclasses(nc)
    return nc


# ---------------- top-level entry ----------------

_CACHE = {}


def kernel(x, edge_index, W1, att_src1, att_dst1, b1, W2, att_src2, att_dst2,
           b2, _trace=False):
    in_maps, meta = _host_prep(x, edge_index, W1, att_src1, att_dst1, b1, W2,
                               att_src2, att_dst2, b2)
    if "prog" not in _CACHE:
        _CACHE["prog"] = _build_program(meta["nvL1"])
    nc = _CACHE["prog"]
    res = run_bass_kernel_spmd(nc, in_maps, list(range(NCORES)), trace=_trace)
    node_of_gid = meta["node_of_gid"]
    out = np.zeros((N, 64), np.float32)
    Z = np.zeros((N, 64), np.float32)
    for c in range(NCORES):
        oc = np.asarray(res.results[c]["out_cat"])
        cols = node_of_gid[c * NLOC:(c + 1) * NLOC]
        valid = cols >= 0
        out[cols[valid]] = oc[valid, 0:64]
        Z[cols[valid]] = oc[valid, 64:128]
    kernel._last_exec_ns = res.exec_time_ns
    kernel._last_res = res
    return (out, Z)
